# revision 1
# baseline (speedup 1.0000x reference)
"""MLA attention kernel for Trainium2 — 8-core tensor-parallel (self-contained).

Sharding: data-parallel over batch (2) x tensor-parallel over head groups
(4 groups of 4 heads) = 8 cores, SPMD (one NEFF, per-core input shards).
Core ci: batch ci//4, heads [4*(ci%4), 4*(ci%4)+4).

Per-core dataflow (everything feature-major "transposed" so the PE never
needs an on-chip transpose):
  x^T tiles via bf16 DMA-transpose (prefetched one chunk ahead)
  kv^T = wkvd.T @ x^T (rmsnorm sum via ones-matmul, scale broadcast via K=1 matmul)
  q_lat^T = wqd.T @ x^T ; q^T = wqu.T @ q_lat^T ; k_nope^T = wkvuk.T @ kv_c^T
  v (token-major) = kv_c^T.T @ wkvuv
  RoPE on rope rows (DVE); host-permuted weight columns group x1/x2 rows
  scores^T[k,q] = k^T.T @ q^T -> exp (ACT, scale folded) -> causal mask (DVE)
  denom[1,q] = ones.T @ E ; out_h^T[v,q] = v.T @ E   (both pipelined 3 deep)
  normalize via K=1 broadcast matmul of 1/denom, deferred one iteration
  out[t, hid] = attn^T.T @ w_out (token-major, contiguous writes)
Host: sums the 4 partial outputs per batch.
"""

import math

import numpy as np
import ml_dtypes

# ---- problem constants (from the reference model) ----
B, S, HID = 2, 2048, 2048
H, D_NOPE, D_ROPE, V_DIM = 16, 128, 64, 128
KV_RANK, Q_RANK = 512, 1536
HEAD_DIM = D_NOPE + D_ROPE
THETA, EPS = 10000.0, 1e-6
NCORES = 8
NH = 4                    # heads per core
T = 512                   # phase-A token chunk
NT = S // T
QC = 512                  # attention query chunk
NQC = S // QC
KH = HID // 128           # 16 k-chunks over HID
RQ = Q_RANK // 128        # 12 chunks over q rank
RKV = KV_RANK // 128      # 4 chunks over kv rank
SCALE = 1.0 / math.sqrt(HEAD_DIM)

_CACHE = {}


def build_nc(taps=False):
    """Build the Bass/Tile program (one NeuronCore, run SPMD on 8)."""
    from contextlib import ExitStack

    import concourse.mybir as mybir
    import concourse.tile as tile
    from concourse import bacc
    from concourse.bass import ds

    dt = mybir.dt
    AF = mybir.ActivationFunctionType
    bf16 = dt.bfloat16
    f32 = dt.float32

    nc = bacc.Bacc(
        "TRN2",
        target_bir_lowering=False,
        debug=False,
        enable_asserts=False,
        num_devices=NCORES,
    )

    # ---- I/O ----
    x_ap = nc.dram_tensor("x", [HID, S // 4], bf16, kind="ExternalInput").ap()
    wqd_ap = nc.dram_tensor("wqd", [HID, Q_RANK], bf16, kind="ExternalInput").ap()
    wqu_ap = nc.dram_tensor("wqu", [Q_RANK, NH * HEAD_DIM], bf16, kind="ExternalInput").ap()
    wkvd_ap = nc.dram_tensor("wkvd", [HID, KV_RANK + D_ROPE], bf16, kind="ExternalInput").ap()
    wkvuk_ap = nc.dram_tensor("wkvuk", [KV_RANK, NH * D_NOPE], bf16, kind="ExternalInput").ap()
    wkvuv_ap = nc.dram_tensor("wkvuv", [KV_RANK, NH * V_DIM], bf16, kind="ExternalInput").ap()
    wout_ap = nc.dram_tensor("wout", [NH * V_DIM, HID], bf16, kind="ExternalInput").ap()
    cos_ap = nc.dram_tensor("cosq", [128, S], bf16, kind="ExternalInput").ap()
    sin_ap = nc.dram_tensor("sinq", [128, S], bf16, kind="ExternalInput").ap()
    mask_ap = nc.dram_tensor("maskt", [128, 1024], bf16, kind="ExternalInput").ap()
    onesc_ap = nc.dram_tensor("ones_col", [128, 1], bf16, kind="ExternalInput").ap()
    onesr_ap = nc.dram_tensor("ones_row", [1, 128], f32, kind="ExternalInput").ap()
    cosl_ap = nc.dram_tensor("cosl", [128, S // 4], bf16, kind="ExternalInput").ap()
    sinl_ap = nc.dram_tensor("sinl", [128, S // 4], bf16, kind="ExternalInput").ap()
    out_ap = nc.dram_tensor("out", [S, HID], f32, kind="ExternalOutput").ap()

    with tile.TileContext(nc) as tc, ExitStack() as ctx:
        const = ctx.enter_context(tc.tile_pool(name="const", bufs=1))
        dram = ctx.enter_context(tc.tile_pool(name="dram", bufs=1, space="DRAM"))
        mm_ps = ctx.enter_context(tc.tile_pool(name="mm_ps", bufs=3, space="PSUM"))
        pv_ps = ctx.enter_context(tc.tile_pool(name="pv_ps", bufs=2, space="PSUM"))
        sm_ps = ctx.enter_context(tc.tile_pool(name="sm_ps", bufs=1, space="PSUM"))
        bc_ps = ctx.enter_context(tc.tile_pool(name="bc_ps", bufs=1, space="PSUM"))

        # ---- resident constants ----
        TL = S // 4  # local token quarter
        workA = tc.alloc_tile_pool(name="workA", bufs=2)

        # local x^T quarter (host pre-transposed, pre-sharded)
        xt = workA.tile([128, KH, TL], bf16, tag="xt", bufs=1)
        for i in range(KH):
            nc.sync.dma_start(out=xt[:, i, :], in_=x_ap[ds(i * 128, 128), :])

        wkvd_sb = const.tile([128, KH, KV_RANK + D_ROPE], bf16, name="wkvd_sb")
        for k in range(KH):
            nc.sync.dma_start(out=wkvd_sb[:, k, :], in_=wkvd_ap[ds(k * 128, 128), :])
        wqd_sb = const.tile([128, KH, Q_RANK], bf16, name="wqd_sb")
        for k in range(KH):
            nc.sync.dma_start(out=wqd_sb[:, k, :], in_=wqd_ap[ds(k * 128, 128), :])
        wqu_sb = const.tile([128, RQ, NH * HEAD_DIM], bf16, name="wqu_sb")
        for r in range(RQ):
            nc.sync.dma_start(out=wqu_sb[:, r, :], in_=wqu_ap[ds(r * 128, 128), :])
        wkvuk_sb = const.tile([128, RKV, NH * D_NOPE], bf16, name="wkvuk_sb")
        for j in range(RKV):
            nc.sync.dma_start(out=wkvuk_sb[:, j, :], in_=wkvuk_ap[ds(j * 128, 128), :])
        wkvuv_sb = const.tile([128, RKV, NH * V_DIM], bf16, name="wkvuv_sb")
        for j in range(RKV):
            nc.sync.dma_start(out=wkvuv_sb[:, j, :], in_=wkvuv_ap[ds(j * 128, 128), :])
        cos_sb = const.tile([128, S], bf16, name="cos_sb")
        nc.sync.dma_start(out=cos_sb[:], in_=cos_ap[:])
        sin_sb = const.tile([128, S], bf16, name="sin_sb")
        nc.sync.dma_start(out=sin_sb[:], in_=sin_ap[:])
        cosl_sb = const.tile([128, TL], bf16, name="cosl_sb")
        nc.sync.dma_start(out=cosl_sb[:], in_=cosl_ap[:])
        sinl_sb = const.tile([128, TL], bf16, name="sinl_sb")
        nc.sync.dma_start(out=sinl_sb[:], in_=sinl_ap[:])
        mask_sb = const.tile([128, 1024], bf16, name="mask_sb")
        nc.sync.dma_start(out=mask_sb[:], in_=mask_ap[:])
        onesc_sb = const.tile([128, 1], bf16, name="onesc_sb")
        nc.sync.dma_start(out=onesc_sb[:], in_=onesc_ap[:])
        onesr_f32 = const.tile([1, 128], f32, name="onesr_f32")
        nc.sync.dma_start(out=onesr_f32[:], in_=onesr_ap[:])
        onesr_sb = const.tile([1, 128], dt.float32r, name="onesr_sb")
        with nc.allow_low_precision(reason="exact ones rounded to f32r"):
            nc.vector.tensor_copy(onesr_sb[:], onesr_f32[:])

        krope_sb = const.tile([64, S], bf16, name="krope_sb")
        eps_sb = const.tile([1, 1], f32, name="eps_sb")
        nc.gpsimd.memset(eps_sb[:], EPS)
        at_sb = const.tile([128, NH, S], bf16, name="at_sb")

        # DRAM spill + collective bounce buffers
        tk = "ExternalOutput" if taps else "Internal"
        qn_dram = dram.tile([128, NH, S], bf16, name="qn_dram", kind=tk)
        qr_dram = dram.tile([64, NH, S], bf16, name="qr_dram", kind=tk)
        kn_dram = dram.tile([128, NH, S], bf16, name="kn_dram", kind=tk)
        v_dram = dram.tile([128, S // 128, NH * V_DIM], bf16, name="v_dram", kind=tk)
        at_dram = dram.tile([128, NH, S], bf16, name="at_dram", kind=tk) if taps else None
        gin_kv = dram.tile([KV_RANK + D_ROPE, TL], bf16, name="gin_kv")
        gout_kv = dram.tile([4, KV_RANK + D_ROPE, TL], bf16, name="gout_kv")
        gin_q = dram.tile([Q_RANK, TL], bf16, name="gin_q")
        gout_q = dram.tile([4, Q_RANK, TL], bf16, name="gout_q")
        GROUPS = [[0, 1, 2, 3], [4, 5, 6, 7]]

        # ================= phase A0: local down-projections =================
        work = workA
        # ---- kv down (local quarter) ----
        kvc_bf = work.tile([128, RKV, TL], bf16, tag="kvc", bufs=1)
        sq_bf = work.tile([128, RKV, TL], bf16, tag="sq", bufs=1)
        for j in range(RKV):
            ps = mm_ps.tile([128, TL], f32, tag="mm")
            for k in range(KH):
                nc.tensor.matmul(
                    ps, wkvd_sb[:, k, ds(j * 128, 128)], xt[:, k, :],
                    start=(k == 0), stop=(k == KH - 1),
                )
            nc.scalar.activation(sq_bf[:, j, :], ps, AF.Square)
            nc.vector.tensor_copy(kvc_bf[:, j, :], ps)
        ms = sm_ps.tile([1, TL], f32, tag="rowps", bufs=2)
        for j in range(RKV):
            nc.tensor.matmul(
                ms, onesc_sb[:], sq_bf[:, j, :],
                start=(j == 0), stop=(j == RKV - 1),
            )
        krp = mm_ps.tile([64, TL], f32, tag="mm")
        for k in range(KH):
            nc.tensor.matmul(
                krp, wkvd_sb[:, k, ds(KV_RANK, D_ROPE)], xt[:, k, :],
                start=(k == 0), stop=(k == KH - 1),
            )
        srt = work.tile([1, TL], f32, tag="srt", bufs=1)
        nc.scalar.activation(srt, ms, AF.Sqrt, bias=eps_sb[:], scale=1.0 / KV_RANK)
        rinv = work.tile([1, TL], dt.float32r, tag="rinv", bufs=1)
        with nc.allow_low_precision(reason="rsqrt scale rounded to f32r for broadcast matmul"):
            nc.vector.reciprocal(rinv, srt)
        # k rope rotate (local quarter, local cos/sin)
        kr_raw = work.tile([64, TL], f32, tag="kr_raw", bufs=1)
        nc.vector.tensor_copy(kr_raw, krp)
        kr_sh = work.tile([64, TL], f32, tag="kr_sh", bufs=1)
        nc.gpsimd.dma_start(out=kr_sh[0:32, :], in_=kr_raw[32:64, :])
        nc.gpsimd.dma_start(out=kr_sh[32:64, :], in_=kr_raw[0:32, :])
        kt1 = work.tile([64, TL], f32, tag="kt1", bufs=1)
        kt2 = work.tile([64, TL], f32, tag="kt2", bufs=1)
        nc.vector.tensor_mul(kt1, kr_raw, cosl_sb[0:64, :])
        nc.vector.tensor_mul(kt2, kr_sh, sinl_sb[0:64, :])
        krl = work.tile([64, TL], bf16, tag="krl", bufs=1)
        nc.vector.tensor_sub(krl[0:32, :], kt1[0:32, :], kt2[0:32, :])
        nc.vector.tensor_add(krl[32:64, :], kt1[32:64, :], kt2[32:64, :])
        # kvcn = kvc * rsqrt(ms)
        rbc_ps = bc_ps.tile([128, TL], f32, tag="bc")
        nc.tensor.matmul(rbc_ps, onesr_sb[:], rinv[:], start=True, stop=True)
        rbc = work.tile([128, TL], f32, tag="rbc", bufs=1)
        nc.vector.tensor_copy(rbc, rbc_ps)
        kvcn = work.tile([128, RKV, TL], bf16, tag="kvcn", bufs=1)
        for j in range(RKV):
            nc.vector.tensor_mul(kvcn[:, j, :], kvc_bf[:, j, :], rbc)
        # pack + gather kv latents
        for j in range(RKV):
            nc.gpsimd.dma_start(out=gin_kv[ds(j * 128, 128), :], in_=kvcn[:, j, :])
        nc.gpsimd.dma_start(out=gin_kv[ds(KV_RANK, D_ROPE), :], in_=krl[:])
        nc.gpsimd.collective_compute(
            "AllGather", mybir.AluOpType.bypass, replica_groups=GROUPS,
            ins=[gin_kv.opt()], outs=[gout_kv.opt()],
        )

        # ---- q down (local quarter) ----
        qlat = work.tile([128, RQ, TL], bf16, tag="qlat", bufs=1)
        for m in range(RQ):
            ps = mm_ps.tile([128, TL], f32, tag="mm")
            for k in range(KH):
                nc.tensor.matmul(
                    ps, wqd_sb[:, k, ds(m * 128, 128)], xt[:, k, :],
                    start=(k == 0), stop=(k == KH - 1),
                )
            nc.vector.tensor_copy(qlat[:, m, :], ps)
            nc.gpsimd.dma_start(out=gin_q[ds(m * 128, 128), :], in_=qlat[:, m, :])
        nc.gpsimd.collective_compute(
            "AllGather", mybir.AluOpType.bypass, replica_groups=GROUPS,
            ins=[gin_q.opt()], outs=[gout_q.opt()],
        )

        # krope full from gathered blocks
        for c in range(4):
            nc.sync.dma_start(out=krope_sb[:, ds(c * TL, TL)], in_=gout_kv[c, ds(KV_RANK, D_ROPE), :])

        workA.release()
        workA1 = tc.alloc_tile_pool(name="workA1", bufs=2)
        work = workA1

        # ================= phase A1: kv up-projections per chunk =================
        for c in range(NT):
            csl = ds(c * T, T)
            kvg = work.tile([128, RKV, T], bf16, tag="kvg", bufs=2)
            for j in range(RKV):
                nc.sync.dma_start(out=kvg[:, j, :], in_=gout_kv[c, ds(j * 128, 128), :])
            for m in range(NH):
                ps = mm_ps.tile([128, T], f32, tag="mm")
                for j in range(RKV):
                    nc.tensor.matmul(
                        ps, wkvuk_sb[:, j, ds(m * 128, 128)], kvg[:, j, :],
                        start=(j == 0), stop=(j == RKV - 1),
                    )
                knt = work.tile([128, T], bf16, tag="cast", bufs=3)
                nc.vector.tensor_copy(knt, ps)
                nc.gpsimd.dma_start(out=kn_dram[:, m, csl], in_=knt)
            for s2 in range(T // 128):
                ps = mm_ps.tile([128, NH * V_DIM], f32, tag="mm")
                for j in range(RKV):
                    nc.tensor.matmul(
                        ps, kvg[:, j, ds(s2 * 128, 128)], wkvuv_sb[:, j, :],
                        start=(j == 0), stop=(j == RKV - 1),
                    )
                vt = work.tile([128, NH * V_DIM], bf16, tag="cast", bufs=3)
                nc.vector.tensor_copy(vt, ps)
                nc.gpsimd.dma_start(out=v_dram[:, c * (T // 128) + s2, :], in_=vt)

        # ================= phase A2: q up-projections per chunk =================
        for c in range(NT):
            csl = ds(c * T, T)
            qlg = work.tile([128, RQ, T], bf16, tag="qlg", bufs=2)
            for m in range(RQ):
                nc.sync.dma_start(out=qlg[:, m, :], in_=gout_q[c, ds(m * 128, 128), :])
            for m in range(NH):
                ps = mm_ps.tile([128, T], f32, tag="mm")
                for r in range(RQ):
                    nc.tensor.matmul(
                        ps, wqu_sb[:, r, ds(m * 128, 128)], qlg[:, r, :],
                        start=(r == 0), stop=(r == RQ - 1),
                    )
                qnt = work.tile([128, T], bf16, tag="cast", bufs=3)
                nc.vector.tensor_copy(qnt, ps)
                nc.gpsimd.dma_start(out=qn_dram[:, m, csl], in_=qnt)
            ps1 = mm_ps.tile([128, T], f32, tag="mm")
            for r in range(RQ):
                nc.tensor.matmul(
                    ps1, wqu_sb[:, r, ds(NH * D_NOPE, 128)], qlg[:, r, :],
                    start=(r == 0), stop=(r == RQ - 1),
                )
            ps2 = mm_ps.tile([128, T], f32, tag="mm")
            for r in range(RQ):
                nc.tensor.matmul(
                    ps2, wqu_sb[:, r, ds(NH * D_NOPE + 128, 128)], qlg[:, r, :],
                    start=(r == 0), stop=(r == RQ - 1),
                )
            qa = work.tile([128, T], f32, tag="qa", bufs=1)
            qb = work.tile([128, T], f32, tag="qb", bufs=1)
            nc.vector.tensor_mul(qa, ps1, cos_sb[:, csl])
            nc.vector.tensor_mul(qb, ps2, sin_sb[:, csl])
            y1 = work.tile([128, T], bf16, tag="y1", bufs=2)
            nc.vector.tensor_sub(y1, qa, qb)
            qa2 = work.tile([128, T], f32, tag="qa", bufs=1)
            qb2 = work.tile([128, T], f32, tag="qb", bufs=1)
            nc.vector.tensor_mul(qa2, ps2, cos_sb[:, csl])
            nc.vector.tensor_mul(qb2, ps1, sin_sb[:, csl])
            y2 = work.tile([128, T], bf16, tag="y2", bufs=2)
            nc.vector.tensor_add(y2, qa2, qb2)
            for h in range(NH):
                nc.gpsimd.dma_start(out=qr_dram[0:32, h, csl], in_=y1[ds(32 * h, 32), :])
                nc.gpsimd.dma_start(out=qr_dram[32:64, h, csl], in_=y2[ds(32 * h, 32), :])

        # ================= phase B: attention =================
        workA1.release()
        workB = tc.alloc_tile_pool(name="workB", bufs=2)
        work = workB

        def drain_norm(st):
            # deferred normalize: by now rec (DVE) has long finished
            h_, qsl_, pv_, rec_ = st
            rb2_ps = bc_ps.tile([128, QC], f32, tag="bc")
            nc.tensor.matmul(rb2_ps, onesr_sb[:], rec_[:], start=True, stop=True)
            rbs = work.tile([128, QC], f32, tag="rbs", bufs=2)
            nc.vector.tensor_copy(rbs, rb2_ps)
            nc.vector.tensor_mul(at_sb[:, h_, qsl_], pv_, rbs)
            if taps:
                nc.sync.dma_start(out=at_dram[:, h_, qsl_], in_=at_sb[:, h_, qsl_])

        norm_pend = []
        for qc in range(NQC):
            qsl = ds(qc * QC, QC)
            nkc = 4 * qc + 4
            for h in range(NH):
                qn_t = work.tile([128, QC], bf16, tag="qn", bufs=2)
                nc.sync.dma_start(out=qn_t, in_=qn_dram[:, h, qsl])
                qr_t = work.tile([64, QC], bf16, tag="qr", bufs=2)
                nc.sync.dma_start(out=qr_t, in_=qr_dram[:, h, qsl])
                pv = pv_ps.tile([128, QC], f32, tag="pv")
                den = sm_ps.tile([1, QC], f32, tag="rowps", bufs=2)
                pend = []
                for kc in range(nkc):
                    kn_t = work.tile([128, 128], bf16, tag="kn", bufs=12)
                    nc.sync.dma_start(out=kn_t, in_=kn_dram[:, h, ds(kc * 128, 128)])
                    v_t = work.tile([128, 128], bf16, tag="vt", bufs=12)
                    nc.sync.dma_start(out=v_t, in_=v_dram[:, kc, ds(h * V_DIM, V_DIM)])
                    sps = mm_ps.tile([128, QC], f32, tag="mm")
                    nc.tensor.matmul(sps, kn_t, qn_t, start=True, stop=False)
                    nc.tensor.matmul(
                        sps, krope_sb[:, ds(kc * 128, 128)], qr_t,
                        start=False, stop=True,
                    )
                    E = work.tile([128, QC], bf16, tag="E", bufs=9)
                    nc.scalar.activation(E, sps, AF.Exp, scale=SCALE)
                    dm = kc - 4 * qc
                    if dm >= 0:
                        nc.vector.tensor_mul(E, E, mask_sb[:, ds(512 - 128 * dm, 512)])
                    pend.append((kc, E, v_t))
                    if len(pend) > 6:  # drain den/pv six kc behind the scores
                        pkc, pE, pvt = pend.pop(0)
                        nc.tensor.matmul(den, onesc_sb[:], pE, start=(pkc == 0), stop=False)
                        nc.tensor.matmul(pv, pvt, pE, start=(pkc == 0), stop=False)
                while pend:
                    last = len(pend) == 1
                    pkc, pE, pvt = pend.pop(0)
                    nc.tensor.matmul(den, onesc_sb[:], pE, start=(pkc == 0), stop=last)
                    nc.tensor.matmul(pv, pvt, pE, start=(pkc == 0), stop=last)
                rec = work.tile([1, QC], dt.float32r, tag="rec", bufs=2)
                with nc.allow_low_precision(reason="softmax denom rounded to f32r for broadcast matmul"):
                    nc.vector.reciprocal(rec, den)
                norm_pend.append((h, qsl, pv, rec))
                if len(norm_pend) > 1:
                    drain_norm(norm_pend.pop(0))
        while norm_pend:
            drain_norm(norm_pend.pop(0))

        # ================= phase C: out-projection =================
        workB.release()
        workC = ctx.enter_context(tc.tile_pool(name="workC", bufs=2))
        work = workC
        wo_ts = []
        for n in range(HID // 512):
            wo_t = work.tile([128, NH, 512], bf16, tag="wo", bufs=4)
            for f in range(NH):
                nc.sync.dma_start(
                    out=wo_t[:, f, :], in_=wout_ap[ds(f * 128, 128), ds(n * 512, 512)]
                )
            wo_ts.append(wo_t)
        for n in range(HID // 512):
            wo_t = wo_ts[n]
            for t16 in range(S // 128):
                ps = mm_ps.tile([128, 512], f32, tag="mm")
                for f in range(NH):
                    nc.tensor.matmul(
                        ps, at_sb[:, f, ds(t16 * 128, 128)], wo_t[:, f, :],
                        start=(f == 0), stop=(f == NH - 1),
                    )
                o_t = work.tile([128, 512], f32, tag="ot", bufs=3)
                nc.vector.tensor_copy(o_t, ps)
                nc.sync.dma_start(
                    out=out_ap[ds(t16 * 128, 128), ds(n * 512, 512)], in_=o_t
                )

    nc.compile()
    return nc


def get_nc():
    if "nc" not in _CACHE:
        _CACHE["nc"] = build_nc()
    return _CACHE["nc"]


def host_inputs(x, w_q_down, w_q_up, w_kv_down, kv_norm_w, w_kv_up, w_out):
    """Build the 8 per-core input shards (host-side prep, numpy only)."""
    bf = ml_dtypes.bfloat16
    x = np.asarray(x, np.float32)
    inv = 1.0 / THETA ** (np.arange(0, D_ROPE, 2, dtype=np.float64) / D_ROPE)
    ang = np.arange(S, dtype=np.float64)[:, None] * inv[None, :]      # (S, 32)
    cosq = np.ascontiguousarray(np.tile(np.cos(ang).T, (4, 1))).astype(bf)  # (128, S)
    sinq = np.ascontiguousarray(np.tile(np.sin(ang).T, (4, 1))).astype(bf)
    maskt = (
        np.arange(1024)[None, :] >= (np.arange(128)[:, None] + 512)
    ).astype(bf)
    ones_col = np.ones((128, 1), bf)
    ones_row = np.ones((1, 128), np.float32)
    wkv_eff = np.asarray(w_kv_up, np.float32) * np.asarray(kv_norm_w, np.float32)[:, None]

    xT_bf = [np.ascontiguousarray(x[b].T).astype(bf) for b in range(B)]
    wqd_bf = np.asarray(w_q_down, np.float32).astype(bf)
    wkvd_bf = np.asarray(w_kv_down, np.float32).astype(bf)
    wqu_f = np.asarray(w_q_up, np.float32)
    wout_f = np.asarray(w_out, np.float32)

    in_maps = []
    for ci in range(NCORES):
        b, hg = divmod(ci, 4)
        heads = list(range(NH * hg, NH * hg + NH))
        qu_cols = (
            [h * HEAD_DIM + j for h in heads for j in range(D_NOPE)]
            + [h * HEAD_DIM + D_NOPE + j for h in heads for j in range(32)]
            + [h * HEAD_DIM + D_NOPE + 32 + j for h in heads for j in range(32)]
        )
        kn_cols = [h * (D_NOPE + V_DIM) + j for h in heads for j in range(D_NOPE)]
        v_cols = [h * (D_NOPE + V_DIM) + D_NOPE + j for h in heads for j in range(V_DIM)]
        in_maps.append(
            {
                "x": np.ascontiguousarray(xT_bf[b][:, 512 * hg : 512 * (hg + 1)]),
                "cosl": np.ascontiguousarray(cosq[:, 512 * hg : 512 * (hg + 1)]),
                "sinl": np.ascontiguousarray(sinq[:, 512 * hg : 512 * (hg + 1)]),
                "wqd": wqd_bf,
                "wqu": np.ascontiguousarray(wqu_f[:, qu_cols]).astype(bf),
                "wkvd": wkvd_bf,
                "wkvuk": np.ascontiguousarray(wkv_eff[:, kn_cols]).astype(bf),
                "wkvuv": np.ascontiguousarray(wkv_eff[:, v_cols]).astype(bf),
                "wout": np.ascontiguousarray(
                    wout_f[NH * V_DIM * hg : NH * V_DIM * (hg + 1), :]
                ).astype(bf),
                "cosq": cosq,
                "sinq": sinq,
                "maskt": maskt,
                "ones_col": ones_col,
                "ones_row": ones_row,
            }
        )
    return in_maps


def run(inputs, trace=False, trace_cores=None):
    from concourse.bass_utils import run_bass_kernel_spmd

    nc = get_nc()
    in_maps = host_inputs(**inputs)
    res = run_bass_kernel_spmd(
        nc,
        in_maps,
        core_ids=list(range(NCORES)),
        trace=trace,
        trace_cores=trace_cores,
    )
    out = np.zeros((B, S, HID), np.float32)
    for ci in range(NCORES):
        out[ci // 4] += res.results[ci]["out"]
    return out, res


def kernel(**inputs):
    out, _ = run(inputs, trace=False)
    return out



# revision 7
# speedup vs baseline: 1.1891x; 1.1891x over previous
"""MLA attention kernel for Trainium2 — 8-core tensor-parallel (self-contained).

Sharding: data-parallel over batch (2) x tensor-parallel over head groups
(4 groups of 4 heads) = 8 cores, SPMD (one NEFF, per-core input shards).
Core ci: batch ci//4, heads [4*(ci%4), 4*(ci%4)+4).

v2 layout: all intermediates (k_nope / v / q_nope / q_rope / attention out)
stay resident in SBUF between phases; only the latent gathers bounce through
DRAM (collectives require it).  Softmax denominators and the rmsnorm scale
are broadcast via an all-ones [128,128] stationary matmul so the reciprocal
runs on 128 DVE lanes.  Score PSUM tiles are [128,1024] (2 banks) so one
ACT exp instruction covers two key chunks; the K=64 rope score matmuls for
the two chunks run concurrently in disjoint PE row-groups (partitions 0-63
vs 64-127).  The out-projection is interleaved into the attention loop per
query chunk, output written fp16.
"""

import math

import numpy as np
import ml_dtypes

# ---- problem constants (from the reference model) ----
B, S, HID = 2, 2048, 2048
H, D_NOPE, D_ROPE, V_DIM = 16, 128, 64, 128
KV_RANK, Q_RANK = 512, 1536
HEAD_DIM = D_NOPE + D_ROPE
THETA, EPS = 10000.0, 1e-6
NCORES = 8
NH = 4                    # heads per core
T = 512                   # token chunk
NT = S // T
QC = 512                  # attention query chunk
NQC = S // QC
KH = HID // 128           # 16 k-chunks over HID
RQ = Q_RANK // 128        # 12 chunks over q rank
RKV = KV_RANK // 128      # 4 chunks over kv rank
SCALE = 1.0 / math.sqrt(HEAD_DIM)

_CACHE = {}


def build_nc():
    """Build the Bass/Tile program (one NeuronCore, run SPMD on 8)."""
    from contextlib import ExitStack

    import concourse.mybir as mybir
    import concourse.tile as tile
    from concourse import bacc
    from concourse.bass import ds

    dt = mybir.dt
    AF = mybir.ActivationFunctionType
    bf16 = dt.bfloat16
    f32 = dt.float32
    f16 = dt.float16

    nc = bacc.Bacc(
        "TRN2",
        target_bir_lowering=False,
        debug=False,
        enable_asserts=False,
        num_devices=NCORES,
    )

    # ---- I/O ----
    x_ap = nc.dram_tensor("x", [HID, S // 4], bf16, kind="ExternalInput").ap()
    wqd_ap = nc.dram_tensor("wqd", [HID, Q_RANK], bf16, kind="ExternalInput").ap()
    wqu_ap = nc.dram_tensor("wqu", [Q_RANK, NH * HEAD_DIM], bf16, kind="ExternalInput").ap()
    wkvd_ap = nc.dram_tensor("wkvd", [HID, KV_RANK + D_ROPE], bf16, kind="ExternalInput").ap()
    wkvuk_ap = nc.dram_tensor("wkvuk", [KV_RANK, NH * D_NOPE], bf16, kind="ExternalInput").ap()
    wkvuv_ap = nc.dram_tensor("wkvuv", [KV_RANK, NH * V_DIM], bf16, kind="ExternalInput").ap()
    wout_ap = nc.dram_tensor("wout", [NH * V_DIM, HID], bf16, kind="ExternalInput").ap()
    cos_ap = nc.dram_tensor("cosq", [128, S], bf16, kind="ExternalInput").ap()
    sin_ap = nc.dram_tensor("sinq", [128, S], bf16, kind="ExternalInput").ap()
    mask_ap = nc.dram_tensor("maskt", [128, 1024], bf16, kind="ExternalInput").ap()
    ones_ap = nc.dram_tensor("ones128", [128, 128], bf16, kind="ExternalInput").ap()
    cosl_ap = nc.dram_tensor("cosl", [128, S // 4], bf16, kind="ExternalInput").ap()
    sinl_ap = nc.dram_tensor("sinl", [128, S // 4], bf16, kind="ExternalInput").ap()
    out_ap = nc.dram_tensor("out", [S, HID], f16, kind="ExternalOutput").ap()

    with tile.TileContext(nc) as tc, ExitStack() as ctx:
        # ---- PSUM pools: 2x[128,1024] + 2x[128,512] + 2x[128,512] = 8 banks
        sc_ps = ctx.enter_context(tc.tile_pool(name="sc_ps", bufs=2, space="PSUM"))
        pv_ps = ctx.enter_context(tc.tile_pool(name="pv_ps", bufs=2, space="PSUM"))
        aux_ps = ctx.enter_context(tc.tile_pool(name="aux_ps", bufs=2, space="PSUM"))

        def sc_half(state, idx):
            # rotate [128,1024] sc tiles, handing out 512-wide halves
            if idx % 2 == 0:
                state["t"] = sc_ps.tile([128, 1024], f32, tag="sc", name="sct")
            return state["t"][:, ds((idx % 2) * 512, 512)]

        const = ctx.enter_context(tc.tile_pool(name="const", bufs=1))
        woutp = ctx.enter_context(tc.tile_pool(name="woutp", bufs=1))
        dram = ctx.enter_context(tc.tile_pool(name="dram", bufs=1, space="DRAM"))

        TL = S // 4  # local token quarter

        # ---- up-projection weights (needed in A1/A2; allocated below w1 so the
        # pool stack stays LIFO: w2 outlives w1)
        w2 = tc.alloc_tile_pool(name="w2", bufs=1)
        wkvuk_sb = w2.tile([128, RKV, NH * D_NOPE], bf16, tag="wkvuk")
        wkvuv_sb = w2.tile([128, RKV, NH * V_DIM], bf16, tag="wkvuv")
        wqu_sb = w2.tile([128, RQ, NH * HEAD_DIM], bf16, tag="wqu")

        # ---- phase-A0 weights + x (released after A0; space reused for kv/q SBUF stores)
        w1 = tc.alloc_tile_pool(name="w1", bufs=1)
        xt = w1.tile([128, KH, TL], bf16, tag="xt")
        wkvd_sb = w1.tile([128, KH, KV_RANK + D_ROPE], bf16, tag="wkvd")
        wqd_sb = w1.tile([128, KH, Q_RANK], bf16, tag="wqd")
        # priority DMA order on the sync queue: kv-down operands first
        for k in range(KH):
            nc.sync.dma_start(out=wkvd_sb[:, k, :], in_=wkvd_ap[ds(k * 128, 128), :])
            nc.sync.dma_start(out=xt[:, k, :], in_=x_ap[ds(k * 128, 128), :])
        for k in range(KH):
            nc.sync.dma_start(out=wqd_sb[:, k, :], in_=wqd_ap[ds(k * 128, 128), :])
        for j in range(RKV):
            nc.sync.dma_start(out=wkvuk_sb[:, j, :], in_=wkvuk_ap[ds(j * 128, 128), :])
            nc.sync.dma_start(out=wkvuv_sb[:, j, :], in_=wkvuv_ap[ds(j * 128, 128), :])

        # ---- resident constants
        cos_sb = const.tile([128, S], bf16, name="cos_sb")
        nc.sync.dma_start(out=cos_sb[:], in_=cos_ap[:])
        sin_sb = const.tile([128, S], bf16, name="sin_sb")
        nc.sync.dma_start(out=sin_sb[:], in_=sin_ap[:])
        cosl_sb = const.tile([128, TL], bf16, name="cosl_sb")
        nc.sync.dma_start(out=cosl_sb[:], in_=cosl_ap[:])
        sinl_sb = const.tile([128, TL], bf16, name="sinl_sb")
        nc.sync.dma_start(out=sinl_sb[:], in_=sinl_ap[:])
        mask_sb = const.tile([128, 1024], bf16, name="mask_sb")
        nc.sync.dma_start(out=mask_sb[:], in_=mask_ap[:])
        ones_sb = const.tile([128, 128], bf16, name="ones_sb")
        nc.sync.dma_start(out=ones_sb[:], in_=ones_ap[:])
        for r in range(RQ):
            nc.sync.dma_start(out=wqu_sb[:, r, :], in_=wqu_ap[ds(r * 128, 128), :])
        wout_sb = woutp.tile([128, NH, HID], bf16, tag="wout")
        for f in range(NH):
            nc.sync.dma_start(out=wout_sb[:, f, :], in_=wout_ap[ds(f * 128, 128), :])

        krope2_sb = const.tile([128, S], bf16, name="krope2_sb")
        at_sb = const.tile([128, NH, S], bf16, name="at_sb")
        eps_sb = const.tile([128, 1], f32, name="eps_sb")
        nc.gpsimd.memset(eps_sb[:], EPS)

        # DRAM bounce buffers for the latent gathers
        gin_kv = dram.tile([KV_RANK + D_ROPE, TL], bf16, name="gin_kv")
        gout_kv = dram.tile([4, KV_RANK + D_ROPE, TL], bf16, name="gout_kv")
        gin_q = dram.tile([Q_RANK, TL], bf16, name="gin_q")
        gout_q = dram.tile([4, Q_RANK, TL], bf16, name="gout_q")
        GROUPS = [[0, 1, 2, 3], [4, 5, 6, 7]]

        # ================= phase A0: local down-projections =================
        wa = tc.alloc_tile_pool(name="wa", bufs=2)
        st = {}
        kvc_bf = wa.tile([128, RKV, TL], bf16, tag="kvc", bufs=1)
        sq_bf = wa.tile([128, RKV, TL], bf16, tag="sq", bufs=1)
        ms_ps = aux_ps.tile([128, TL], f32, tag="aux", name="ms_ps")
        kv_chunks = []
        for j in range(RKV):
            ps = sc_half(st, j)
            for k in range(KH):
                nc.tensor.matmul(
                    ps, wkvd_sb[:, k, ds(j * 128, 128)], xt[:, k, :],
                    start=(k == 0), stop=(k == KH - 1),
                )
            nc.scalar.activation(sq_bf[:, j, :], ps, AF.Square)
            nc.vector.tensor_copy(kvc_bf[:, j, :], ps)
            kv_chunks.append(j)
            # lag the ones-matmul one chunk behind so the PE never waits on ACT
            if len(kv_chunks) > 1:
                pj = kv_chunks.pop(0)
                nc.tensor.matmul(
                    ms_ps, ones_sb[:], sq_bf[:, pj, :],
                    start=(pj == 0), stop=False,
                )
        # k-rope raw: 16 matmuls, M=64
        krp_ps = aux_ps.tile([128, TL], f32, tag="aux", name="krp_ps")
        for k in range(KH):
            nc.tensor.matmul(
                krp_ps[0:64, :], wkvd_sb[:, k, ds(KV_RANK, D_ROPE)], xt[:, k, :],
                start=(k == 0), stop=(k == KH - 1),
            )
        while kv_chunks:
            pj = kv_chunks.pop(0)
            nc.tensor.matmul(
                ms_ps, ones_sb[:], sq_bf[:, pj, :],
                start=(pj == 0), stop=(pj == RKV - 1),
            )
        # rinv = 1/sqrt(ms/512 + eps), broadcast across partitions by the ones-matmul
        srt = wa.tile([128, TL], f32, tag="srt", bufs=1)
        nc.scalar.activation(srt, ms_ps, AF.Sqrt, bias=eps_sb[:], scale=1.0 / KV_RANK)
        rinv = wa.tile([128, TL], f32, tag="rinv", bufs=1)
        nc.vector.reciprocal_approx_fast(out=rinv, in_=srt)
        kvcn = wa.tile([128, RKV, TL], bf16, tag="kvcn", bufs=1)
        for j in range(RKV):
            nc.vector.tensor_mul(kvcn[:, j, :], kvc_bf[:, j, :], rinv)
            nc.scalar.dma_start(out=gin_kv[ds(j * 128, 128), :], in_=kvcn[:, j, :])
        # k rope rotate (local quarter, local cos/sin)
        kr_raw = wa.tile([64, TL], f32, tag="kr_raw", bufs=1)
        nc.vector.tensor_copy(kr_raw, krp_ps[0:64, :])
        kr_sh = wa.tile([64, TL], f32, tag="kr_sh", bufs=1)
        nc.scalar.dma_start(out=kr_sh[0:32, :], in_=kr_raw[32:64, :])
        nc.scalar.dma_start(out=kr_sh[32:64, :], in_=kr_raw[0:32, :])
        kt1 = wa.tile([64, TL], f32, tag="kt1", bufs=1)
        kt2 = wa.tile([64, TL], f32, tag="kt2", bufs=1)
        nc.vector.tensor_mul(kt1, kr_raw, cosl_sb[0:64, :])
        nc.vector.tensor_mul(kt2, kr_sh, sinl_sb[0:64, :])
        krl = wa.tile([64, TL], bf16, tag="krl", bufs=1)
        nc.vector.tensor_sub(krl[0:32, :], kt1[0:32, :], kt2[0:32, :])
        nc.vector.tensor_add(krl[32:64, :], kt1[32:64, :], kt2[32:64, :])
        nc.scalar.dma_start(out=gin_kv[ds(KV_RANK, D_ROPE), :], in_=krl[:])
        nc.gpsimd.collective_compute(
            "AllGather", mybir.AluOpType.bypass, replica_groups=GROUPS,
            ins=[gin_kv.opt()], outs=[gout_kv.opt()],
        )

        # ---- q down (local quarter) ----
        qlat = wa.tile([128, RQ, TL], bf16, tag="qlat", bufs=1)
        for m in range(RQ):
            ps = sc_half(st, m)
            for k in range(KH):
                nc.tensor.matmul(
                    ps, wqd_sb[:, k, ds(m * 128, 128)], xt[:, k, :],
                    start=(k == 0), stop=(k == KH - 1),
                )
            nc.vector.tensor_copy(qlat[:, m, :], ps)
            nc.scalar.dma_start(out=gin_q[ds(m * 128, 128), :], in_=qlat[:, m, :])
        nc.gpsimd.collective_compute(
            "AllGather", mybir.AluOpType.bypass, replica_groups=GROUPS,
            ins=[gin_q.opt()], outs=[gout_q.opt()],
        )

        # krope full (duplicated on partitions 64-127 for row-group packing)
        for c in range(NT):
            nc.scalar.dma_start(
                out=krope2_sb[0:64, ds(c * TL, TL)], in_=gout_kv[c, ds(KV_RANK, D_ROPE), :]
            )
            nc.scalar.dma_start(
                out=krope2_sb[64:128, ds(c * TL, TL)], in_=gout_kv[c, ds(KV_RANK, D_ROPE), :]
            )

        wa.release()
        w1.release()
        # SBUF-resident intermediates (reuse w1's region)
        kvsb = tc.alloc_tile_pool(name="kvsb", bufs=1)
        kn_sb = kvsb.tile([128, NH, S], bf16, tag="kn")
        v_sb = kvsb.tile([128, S // 128, NH * V_DIM], bf16, tag="v")
        qn_sb = kvsb.tile([128, NH, S], bf16, tag="qn")
        qr2_sb = kvsb.tile([128, NH, S], bf16, tag="qr2")
        wb = tc.alloc_tile_pool(name="wb", bufs=2)

        # ================= phase A1: kv up-projections per chunk =================
        for c in range(NT):
            csl = ds(c * T, T)
            kvg = wb.tile([128, RKV, T], bf16, tag="kvg", bufs=2)
            for j in range(RKV):
                nc.scalar.dma_start(out=kvg[:, j, :], in_=gout_kv[c, ds(j * 128, 128), :])
            for m in range(NH):
                ps = sc_half(st, m)
                for j in range(RKV):
                    nc.tensor.matmul(
                        ps, wkvuk_sb[:, j, ds(m * 128, 128)], kvg[:, j, :],
                        start=(j == 0), stop=(j == RKV - 1),
                    )
                nc.vector.tensor_copy(kn_sb[:, m, csl], ps)
            for s2 in range(T // 128):
                ps = sc_half(st, s2)
                for j in range(RKV):
                    nc.tensor.matmul(
                        ps, kvg[:, j, ds(s2 * 128, 128)], wkvuv_sb[:, j, :],
                        start=(j == 0), stop=(j == RKV - 1),
                    )
                nc.vector.tensor_copy(v_sb[:, c * (T // 128) + s2, :], ps)

        # ================= phase A2: q up-projections per chunk =================
        for c in range(NT):
            csl = ds(c * T, T)
            qlg = wb.tile([128, RQ, T], bf16, tag="qlg", bufs=2)
            for m in range(RQ):
                nc.scalar.dma_start(out=qlg[:, m, :], in_=gout_q[c, ds(m * 128, 128), :])
            for m in range(NH):
                ps = sc_half(st, m)
                for r in range(RQ):
                    nc.tensor.matmul(
                        ps, wqu_sb[:, r, ds(m * 128, 128)], qlg[:, r, :],
                        start=(r == 0), stop=(r == RQ - 1),
                    )
                nc.vector.tensor_copy(qn_sb[:, m, csl], ps)
            sct = sc_ps.tile([128, 1024], f32, tag="sc", name="sct_rope")
            ps1, ps2 = sct[:, 0:512], sct[:, 512:1024]
            for r in range(RQ):
                nc.tensor.matmul(
                    ps1, wqu_sb[:, r, ds(NH * D_NOPE, 128)], qlg[:, r, :],
                    start=(r == 0), stop=(r == RQ - 1),
                )
            for r in range(RQ):
                nc.tensor.matmul(
                    ps2, wqu_sb[:, r, ds(NH * D_NOPE + 128, 128)], qlg[:, r, :],
                    start=(r == 0), stop=(r == RQ - 1),
                )
            qa = wb.tile([128, T], f32, tag="qa", bufs=1)
            qb = wb.tile([128, T], f32, tag="qb", bufs=1)
            nc.vector.tensor_mul(qa, ps1, cos_sb[:, csl])
            nc.vector.tensor_mul(qb, ps2, sin_sb[:, csl])
            y1 = wb.tile([128, T], bf16, tag="y1", bufs=2)
            nc.vector.tensor_sub(y1, qa, qb)
            qa2 = wb.tile([128, T], f32, tag="qa", bufs=1)
            qb2 = wb.tile([128, T], f32, tag="qb", bufs=1)
            nc.vector.tensor_mul(qa2, ps2, cos_sb[:, csl])
            nc.vector.tensor_mul(qb2, ps1, sin_sb[:, csl])
            y2 = wb.tile([128, T], bf16, tag="y2", bufs=2)
            nc.vector.tensor_add(y2, qa2, qb2)
            # assemble per-head [x1(32); x2(32)] rope layout, duplicated at 64-127
            for h in range(NH):
                nc.scalar.dma_start(out=qr2_sb[0:32, h, csl], in_=y1[ds(32 * h, 32), :])
                nc.scalar.dma_start(out=qr2_sb[32:64, h, csl], in_=y2[ds(32 * h, 32), :])
                nc.scalar.dma_start(out=qr2_sb[64:96, h, csl], in_=y1[ds(32 * h, 32), :])
                nc.scalar.dma_start(out=qr2_sb[96:128, h, csl], in_=y2[ds(32 * h, 32), :])

        # ================= phase B + C: attention with interleaved out-proj =====
        wb.release()
        wc = tc.alloc_tile_pool(name="wc", bufs=2)

        norm_pend = []

        def drain_norm(stn):
            h_, qsl_, pv_, eacc_ = stn
            eacc_bf = wc.tile([128, QC], bf16, tag="eaccb", bufs=2)
            nc.vector.tensor_copy(eacc_bf, eacc_)
            den_ps = aux_ps.tile([128, QC], f32, tag="aux", name="den_ps")
            nc.tensor.matmul(den_ps, ones_sb[:], eacc_bf, start=True, stop=True)
            rec = wc.tile([128, QC], f32, tag="rec", bufs=2)
            nc.vector.reciprocal_approx_fast(out=rec, in_=den_ps)
            nc.vector.tensor_mul(at_sb[:, h_, qsl_], pv_, rec)

        for qc in range(NQC):
            qsl = ds(qc * QC, QC)
            nkc = 4 * qc + 4
            for h in range(NH):
                pv = pv_ps.tile([128, QC], f32, tag="pv")
                eacc = wc.tile([128, QC], f32, tag="eacc", bufs=2)
                pend = []
                for t in range(nkc // 2):
                    kcA, kcB = 2 * t, 2 * t + 1
                    sct = sc_ps.tile([128, 1024], f32, tag="sc", name="sct_b")
                    nc.tensor.matmul(
                        sct[:, 0:512], kn_sb[:, h, ds(kcA * 128, 128)], qn_sb[:, h, qsl],
                        start=True, stop=False,
                    )
                    nc.tensor.matmul(
                        sct[:, 512:1024], kn_sb[:, h, ds(kcB * 128, 128)], qn_sb[:, h, qsl],
                        start=True, stop=False,
                    )
                    # the two K=64 rope matmuls land in disjoint row-groups -> concurrent
                    nc.tensor.matmul(
                        sct[:, 0:512], krope2_sb[0:64, ds(kcA * 128, 128)],
                        qr2_sb[0:64, h, qsl], start=False, stop=True,
                    )
                    nc.tensor.matmul(
                        sct[:, 512:1024], krope2_sb[64:128, ds(kcB * 128, 128)],
                        qr2_sb[64:128, h, qsl], start=False, stop=True,
                    )
                    E = wc.tile([128, 1024], bf16, tag="E", bufs=4)
                    nc.scalar.activation(E, sct, AF.Exp, scale=SCALE)
                    dA, dB = kcA - 4 * qc, kcB - 4 * qc
                    if dA >= 0:
                        nc.vector.tensor_mul(
                            E[:, 0:512], E[:, 0:512], mask_sb[:, ds(512 - 128 * dA, 512)]
                        )
                    if dB >= 0:
                        nc.vector.tensor_mul(
                            E[:, 512:1024], E[:, 512:1024], mask_sb[:, ds(512 - 128 * dB, 512)]
                        )
                    if t == 0:
                        nc.vector.tensor_copy(eacc, E[:, 0:512])
                    else:
                        nc.vector.tensor_add(eacc, eacc, E[:, 0:512])
                    nc.vector.tensor_add(eacc, eacc, E[:, 512:1024])
                    pend.append((t, E))
                    if len(pend) > 1:
                        pt, pE = pend.pop(0)
                        nc.tensor.matmul(
                            pv, v_sb[:, 2 * pt, ds(h * V_DIM, V_DIM)], pE[:, 0:512],
                            start=(pt == 0), stop=False,
                        )
                        nc.tensor.matmul(
                            pv, v_sb[:, 2 * pt + 1, ds(h * V_DIM, V_DIM)], pE[:, 512:1024],
                            start=False, stop=False,
                        )
                while pend:
                    pt, pE = pend.pop(0)
                    last = not pend
                    nc.tensor.matmul(
                        pv, v_sb[:, 2 * pt, ds(h * V_DIM, V_DIM)], pE[:, 0:512],
                        start=(pt == 0), stop=False,
                    )
                    nc.tensor.matmul(
                        pv, v_sb[:, 2 * pt + 1, ds(h * V_DIM, V_DIM)], pE[:, 512:1024],
                        start=False, stop=last,
                    )
                norm_pend.append((h, qsl, pv, eacc))
                if len(norm_pend) > 1:
                    drain_norm(norm_pend.pop(0))
            while norm_pend:
                drain_norm(norm_pend.pop(0))
            # ---- out-projection for this qc's 4 token blocks ----
            for t16 in range(qc * 4, qc * 4 + 4):
                for n in range(HID // 512):
                    ps = sc_half(st, n)
                    for f in range(NH):
                        nc.tensor.matmul(
                            ps, at_sb[:, f, ds(t16 * 128, 128)], wout_sb[:, f, ds(n * 512, 512)],
                            start=(f == 0), stop=(f == NH - 1),
                        )
                    o_t = wc.tile([128, 512], f16, tag="ot", bufs=3)
                    nc.vector.tensor_copy(o_t, ps)
                    nc.sync.dma_start(
                        out=out_ap[ds(t16 * 128, 128), ds(n * 512, 512)], in_=o_t
                    )

        wc.release()
        kvsb.release()
        w2.release()

    nc.compile()
    return nc


def get_nc():
    if "nc" not in _CACHE:
        _CACHE["nc"] = build_nc()
    return _CACHE["nc"]


def host_inputs(x, w_q_down, w_q_up, w_kv_down, kv_norm_w, w_kv_up, w_out):
    """Build the 8 per-core input shards (host-side prep, numpy only)."""
    bf = ml_dtypes.bfloat16
    x = np.asarray(x, np.float32)
    inv = 1.0 / THETA ** (np.arange(0, D_ROPE, 2, dtype=np.float64) / D_ROPE)
    ang = np.arange(S, dtype=np.float64)[:, None] * inv[None, :]      # (S, 32)
    cosq = np.ascontiguousarray(np.tile(np.cos(ang).T, (4, 1))).astype(bf)  # (128, S)
    sinq = np.ascontiguousarray(np.tile(np.sin(ang).T, (4, 1))).astype(bf)
    maskt = (
        np.arange(1024)[None, :] >= (np.arange(128)[:, None] + 512)
    ).astype(bf)
    ones128 = np.ones((128, 128), bf)
    wkv_eff = np.asarray(w_kv_up, np.float32) * np.asarray(kv_norm_w, np.float32)[:, None]

    xT_bf = [np.ascontiguousarray(x[b].T).astype(bf) for b in range(B)]
    wqd_bf = np.asarray(w_q_down, np.float32).astype(bf)
    wkvd_bf = np.asarray(w_kv_down, np.float32).astype(bf)
    wqu_f = np.asarray(w_q_up, np.float32)
    wout_f = np.asarray(w_out, np.float32)

    in_maps = []
    for ci in range(NCORES):
        b, hg = divmod(ci, 4)
        heads = list(range(NH * hg, NH * hg + NH))
        qu_cols = (
            [h * HEAD_DIM + j for h in heads for j in range(D_NOPE)]
            + [h * HEAD_DIM + D_NOPE + j for h in heads for j in range(32)]
            + [h * HEAD_DIM + D_NOPE + 32 + j for h in heads for j in range(32)]
        )
        kn_cols = [h * (D_NOPE + V_DIM) + j for h in heads for j in range(D_NOPE)]
        v_cols = [h * (D_NOPE + V_DIM) + D_NOPE + j for h in heads for j in range(V_DIM)]
        in_maps.append(
            {
                "x": np.ascontiguousarray(xT_bf[b][:, 512 * hg : 512 * (hg + 1)]),
                "cosl": np.ascontiguousarray(cosq[:, 512 * hg : 512 * (hg + 1)]),
                "sinl": np.ascontiguousarray(sinq[:, 512 * hg : 512 * (hg + 1)]),
                "wqd": wqd_bf,
                "wqu": np.ascontiguousarray(wqu_f[:, qu_cols]).astype(bf),
                "wkvd": wkvd_bf,
                "wkvuk": np.ascontiguousarray(wkv_eff[:, kn_cols]).astype(bf),
                "wkvuv": np.ascontiguousarray(wkv_eff[:, v_cols]).astype(bf),
                "wout": np.ascontiguousarray(
                    wout_f[NH * V_DIM * hg : NH * V_DIM * (hg + 1), :]
                ).astype(bf),
                "cosq": cosq,
                "sinq": sinq,
                "maskt": maskt,
                "ones128": ones128,
            }
        )
    return in_maps


def run(inputs, trace=False, trace_cores=None):
    from concourse.bass_utils import run_bass_kernel_spmd

    nc = get_nc()
    in_maps = host_inputs(**inputs)
    res = run_bass_kernel_spmd(
        nc,
        in_maps,
        core_ids=list(range(NCORES)),
        trace=trace,
        trace_cores=trace_cores,
    )
    out = np.zeros((B, S, HID), np.float32)
    for ci in range(NCORES):
        out[ci // 4] += res.results[ci]["out"].astype(np.float32)
    return out, res


def kernel(**inputs):
    out, _ = run(inputs, trace=False)
    return out


# revision 16
# speedup vs baseline: 1.2160x; 1.0226x over previous
"""MLA attention kernel for Trainium2 — 8-core tensor-parallel (self-contained).

Sharding: data-parallel over batch (2) x tensor-parallel over head groups
(4 groups of 4 heads) = 8 cores, SPMD (one NEFF, per-core input shards).
Core ci: batch ci//4, heads [4*(ci%4), 4*(ci%4)+4).

v2 layout: all intermediates (k_nope / v / q_nope / q_rope / attention out)
stay resident in SBUF between phases; only the latent gathers bounce through
DRAM (collectives require it).  Softmax denominators and the rmsnorm scale
are broadcast via an all-ones [128,128] stationary matmul so the reciprocal
runs on 128 DVE lanes.  Score PSUM tiles are [128,1024] (2 banks) so one
ACT exp instruction covers two key chunks; the K=64 rope score matmuls for
the two chunks run concurrently in disjoint PE row-groups (partitions 0-63
vs 64-127).  The out-projection is interleaved into the attention loop per
query chunk, output written fp16.
"""

import math

import numpy as np
import ml_dtypes

# ---- problem constants (from the reference model) ----
B, S, HID = 2, 2048, 2048
H, D_NOPE, D_ROPE, V_DIM = 16, 128, 64, 128
KV_RANK, Q_RANK = 512, 1536
HEAD_DIM = D_NOPE + D_ROPE
THETA, EPS = 10000.0, 1e-6
NCORES = 8
NH = 4                    # heads per core
T = 512                   # token chunk
NT = S // T
QC = 512                  # attention query chunk
NQC = S // QC
KH = HID // 128           # 16 k-chunks over HID
RQ = Q_RANK // 128        # 12 chunks over q rank
RKV = KV_RANK // 128      # 4 chunks over kv rank
SCALE = 1.0 / math.sqrt(HEAD_DIM)

_CACHE = {}


def build_nc():
    """Build the Bass/Tile program (one NeuronCore, run SPMD on 8)."""
    from contextlib import ExitStack

    import concourse.mybir as mybir
    import concourse.tile as tile
    from concourse import bacc
    from concourse.bass import ds

    dt = mybir.dt
    AF = mybir.ActivationFunctionType
    bf16 = dt.bfloat16
    f32 = dt.float32
    f16 = dt.float16

    nc = bacc.Bacc(
        "TRN2",
        target_bir_lowering=False,
        debug=False,
        enable_asserts=False,
        num_devices=NCORES,
    )

    # ---- I/O ----
    x_ap = nc.dram_tensor("x", [HID, S // 4], bf16, kind="ExternalInput").ap()
    wqd_ap = nc.dram_tensor("wqd", [HID, Q_RANK], bf16, kind="ExternalInput").ap()
    wqu_ap = nc.dram_tensor("wqu", [Q_RANK, NH * HEAD_DIM], bf16, kind="ExternalInput").ap()
    wkvd_ap = nc.dram_tensor("wkvd", [HID, KV_RANK + D_ROPE], bf16, kind="ExternalInput").ap()
    wkvuk_ap = nc.dram_tensor("wkvuk", [KV_RANK, NH * D_NOPE], bf16, kind="ExternalInput").ap()
    wkvuv_ap = nc.dram_tensor("wkvuv", [KV_RANK, NH * V_DIM], bf16, kind="ExternalInput").ap()
    wout_ap = nc.dram_tensor("wout", [NH * V_DIM, HID], bf16, kind="ExternalInput").ap()
    cos_ap = nc.dram_tensor("cosq", [128, S], bf16, kind="ExternalInput").ap()
    sin_ap = nc.dram_tensor("sinq", [128, S], bf16, kind="ExternalInput").ap()
    mask_ap = nc.dram_tensor("maskp", [128, 2, 1024], bf16, kind="ExternalInput").ap()
    ones_ap = nc.dram_tensor("ones128", [128, 128], bf16, kind="ExternalInput").ap()
    ident_ap = nc.dram_tensor("ident128", [128, 128], bf16, kind="ExternalInput").ap()
    cosl_ap = nc.dram_tensor("cosl", [128, S // 4], bf16, kind="ExternalInput").ap()
    sinl_ap = nc.dram_tensor("sinl", [128, S // 4], bf16, kind="ExternalInput").ap()
    out_ap = nc.dram_tensor("out", [S, HID], f16, kind="ExternalOutput").ap()

    with tile.TileContext(nc) as tc, ExitStack() as ctx:
        # ---- PSUM pools: 2x[128,1024] + 2x[128,512] + 2x[128,512] = 8 banks
        sc_ps = ctx.enter_context(tc.tile_pool(name="sc_ps", bufs=2, space="PSUM"))
        pv_ps = ctx.enter_context(tc.tile_pool(name="pv_ps", bufs=2, space="PSUM"))
        aux_ps = ctx.enter_context(tc.tile_pool(name="aux_ps", bufs=2, space="PSUM"))

        def sc_half(state, idx):
            # rotate [128,1024] sc tiles, handing out 512-wide halves
            if idx % 2 == 0:
                state["t"] = sc_ps.tile([128, 1024], f32, tag="sc", name="sct")
            return state["t"][:, ds((idx % 2) * 512, 512)]

        const = ctx.enter_context(tc.tile_pool(name="const", bufs=1))
        woutp = ctx.enter_context(tc.tile_pool(name="woutp", bufs=1))
        dram = ctx.enter_context(tc.tile_pool(name="dram", bufs=1, space="DRAM"))

        TL = S // 4  # local token quarter

        # ---- up-projection weights (needed in A1/A2; allocated below w1 so the
        # pool stack stays LIFO: w2 outlives w1)
        w2 = tc.alloc_tile_pool(name="w2", bufs=1)
        wkvuk_sb = w2.tile([128, RKV, NH * D_NOPE], bf16, tag="wkvuk")
        wkvuv_sb = w2.tile([128, RKV, NH * V_DIM], bf16, tag="wkvuv")
        wqu_sb = w2.tile([128, RQ, NH * HEAD_DIM], bf16, tag="wqu")

        # ---- phase-A0 weights + x (released after A0; space reused for kv/q SBUF stores)
        w1 = tc.alloc_tile_pool(name="w1", bufs=1)
        xt = w1.tile([128, KH, TL], bf16, tag="xt")
        wkvd_sb = w1.tile([128, KH, KV_RANK + D_ROPE], bf16, tag="wkvd")
        wqd_sb = w1.tile([128, KH, Q_RANK], bf16, tag="wqd")
        # priority DMA order, split across the two HWDGE queues (sync+scalar):
        # kv-down operands first, then wqd, then everything else.
        def ld(i, **kw):
            (nc.sync if i % 2 == 0 else nc.scalar).dma_start(**kw)

        for k in range(KH):
            ld(k, out=wkvd_sb[:, k, :], in_=wkvd_ap[ds(k * 128, 128), :])
            ld(k + 1, out=xt[:, k, :], in_=x_ap[ds(k * 128, 128), :])
        for k in range(KH):
            ld(k, out=wqd_sb[:, k, :], in_=wqd_ap[ds(k * 128, 128), :])
        for j in range(RKV):
            ld(j, out=wkvuk_sb[:, j, :], in_=wkvuk_ap[ds(j * 128, 128), :])
            ld(j + 1, out=wkvuv_sb[:, j, :], in_=wkvuv_ap[ds(j * 128, 128), :])

        # ---- resident constants
        cos_sb = const.tile([128, S], bf16, name="cos_sb")
        ld(0, out=cos_sb[:], in_=cos_ap[:])
        sin_sb = const.tile([128, S], bf16, name="sin_sb")
        ld(1, out=sin_sb[:], in_=sin_ap[:])
        cosl_sb = const.tile([128, TL], bf16, name="cosl_sb")
        ld(0, out=cosl_sb[:], in_=cosl_ap[:])
        sinl_sb = const.tile([128, TL], bf16, name="sinl_sb")
        ld(1, out=sinl_sb[:], in_=sinl_ap[:])
        mask_sb = const.tile([128, 2, 1024], bf16, name="mask_sb")
        ld(0, out=mask_sb[:], in_=mask_ap[:])
        ones_sb = const.tile([128, 128], bf16, name="ones_sb")
        ld(1, out=ones_sb[:], in_=ones_ap[:])
        ident_sb = const.tile([128, 128], bf16, name="ident_sb")
        ld(0, out=ident_sb[:], in_=ident_ap[:])
        for r in range(RQ):
            ld(r, out=wqu_sb[:, r, :], in_=wqu_ap[ds(r * 128, 128), :])
        wout_sb = woutp.tile([128, NH, HID], bf16, tag="wout")
        for f in range(NH):
            ld(f, out=wout_sb[:, f, :], in_=wout_ap[ds(f * 128, 128), :])

        krope2_sb = const.tile([128, S], bf16, name="krope2_sb")
        at_sb = const.tile([128, NH, S], bf16, name="at_sb")
        eps_sb = const.tile([128, 1], f32, name="eps_sb")
        nc.gpsimd.memset(eps_sb[:], EPS)

        # DRAM bounce buffers for the latent gathers
        gin_kv = dram.tile([KV_RANK + D_ROPE, TL], bf16, name="gin_kv")
        gout_kv = dram.tile([4, KV_RANK + D_ROPE, TL], bf16, name="gout_kv")
        gin_q = dram.tile([Q_RANK, TL], bf16, name="gin_q")
        gout_q1 = dram.tile([4, Q_RANK // 2, TL], bf16, name="gout_q1")
        gout_q2 = dram.tile([4, Q_RANK // 2, TL], bf16, name="gout_q2")
        GROUPS = [[0, 1, 2, 3], [4, 5, 6, 7]]

        # ================= phase A0: local down-projections =================
        wa = tc.alloc_tile_pool(name="wa", bufs=2)
        st = {}
        kvc_bf = wa.tile([128, RKV, TL], bf16, tag="kvc", bufs=1)
        sq_bf = wa.tile([128, RKV, TL], bf16, tag="sq", bufs=1)
        ms_ps = aux_ps.tile([128, TL], f32, tag="aux", name="ms_ps")
        kv_chunks = []
        for j in range(RKV):
            ps = sc_half(st, j)
            for k in range(KH):
                nc.tensor.matmul(
                    ps, wkvd_sb[:, k, ds(j * 128, 128)], xt[:, k, :],
                    start=(k == 0), stop=(k == KH - 1),
                )
            nc.scalar.activation(sq_bf[:, j, :], ps, AF.Square)
            nc.vector.tensor_copy(kvc_bf[:, j, :], ps)
            kv_chunks.append(j)
            # lag the ones-matmul one chunk behind so the PE never waits on ACT
            if len(kv_chunks) > 1:
                pj = kv_chunks.pop(0)
                nc.tensor.matmul(
                    ms_ps, ones_sb[:], sq_bf[:, pj, :],
                    start=(pj == 0), stop=False,
                )
        # k-rope raw: 16 matmuls, M=64
        krp_ps = aux_ps.tile([128, TL], f32, tag="aux", name="krp_ps")
        for k in range(KH):
            nc.tensor.matmul(
                krp_ps[0:64, :], wkvd_sb[:, k, ds(KV_RANK, D_ROPE)], xt[:, k, :],
                start=(k == 0), stop=(k == KH - 1),
            )
        while kv_chunks:
            pj = kv_chunks.pop(0)
            nc.tensor.matmul(
                ms_ps, ones_sb[:], sq_bf[:, pj, :],
                start=(pj == 0), stop=(pj == RKV - 1),
            )
        # rinv = 1/sqrt(ms/512 + eps), broadcast across partitions by the ones-matmul
        srt = wa.tile([128, TL], f32, tag="srt", bufs=1)
        nc.scalar.activation(srt, ms_ps, AF.Sqrt, bias=eps_sb[:], scale=1.0 / KV_RANK)
        rinv = wa.tile([128, TL], f32, tag="rinv", bufs=1)
        nc.vector.reciprocal_approx_fast(out=rinv, in_=srt)
        kvcn = wa.tile([128, RKV, TL], bf16, tag="kvcn", bufs=1)
        for j in range(RKV):
            nc.vector.tensor_mul(kvcn[:, j, :], kvc_bf[:, j, :], rinv)
            nc.scalar.dma_start(out=gin_kv[ds(j * 128, 128), :], in_=kvcn[:, j, :])
        # k rope rotate (local quarter, local cos/sin)
        kr_raw = wa.tile([64, TL], f32, tag="kr_raw", bufs=1)
        nc.vector.tensor_copy(kr_raw, krp_ps[0:64, :])
        kr_sh = wa.tile([64, TL], f32, tag="kr_sh", bufs=1)
        nc.scalar.dma_start(out=kr_sh[0:32, :], in_=kr_raw[32:64, :])
        nc.scalar.dma_start(out=kr_sh[32:64, :], in_=kr_raw[0:32, :])
        kt1 = wa.tile([64, TL], f32, tag="kt1", bufs=1)
        kt2 = wa.tile([64, TL], f32, tag="kt2", bufs=1)
        nc.vector.tensor_mul(kt1, kr_raw, cosl_sb[0:64, :])
        nc.vector.tensor_mul(kt2, kr_sh, sinl_sb[0:64, :])
        krl = wa.tile([64, TL], bf16, tag="krl", bufs=1)
        nc.vector.tensor_sub(krl[0:32, :], kt1[0:32, :], kt2[0:32, :])
        nc.vector.tensor_add(krl[32:64, :], kt1[32:64, :], kt2[32:64, :])
        nc.scalar.dma_start(out=gin_kv[ds(KV_RANK, D_ROPE), :], in_=krl[:])
        nc.gpsimd.collective_compute(
            "AllGather", mybir.AluOpType.bypass, replica_groups=GROUPS,
            ins=[gin_kv.opt()], outs=[gout_kv.opt()],
        )

        # ---- q down (local quarter); gather in two halves so the second
        # transfer overlaps with A1 ----
        qlat = wa.tile([128, RQ, TL], bf16, tag="qlat", bufs=1)
        for m in range(RQ):
            ps = sc_half(st, m)
            for k in range(KH):
                nc.tensor.matmul(
                    ps, wqd_sb[:, k, ds(m * 128, 128)], xt[:, k, :],
                    start=(k == 0), stop=(k == KH - 1),
                )
            nc.vector.tensor_copy(qlat[:, m, :], ps)
            nc.scalar.dma_start(out=gin_q[ds(m * 128, 128), :], in_=qlat[:, m, :])
            if m == RQ // 2 - 1:
                nc.gpsimd.collective_compute(
                    "AllGather", mybir.AluOpType.bypass, replica_groups=GROUPS,
                    ins=[gin_q[ds(0, Q_RANK // 2), :].opt()], outs=[gout_q1.opt()],
                )
        nc.gpsimd.collective_compute(
            "AllGather", mybir.AluOpType.bypass, replica_groups=GROUPS,
            ins=[gin_q[ds(Q_RANK // 2, Q_RANK // 2), :].opt()], outs=[gout_q2.opt()],
        )

        # krope full (duplicated on partitions 64-127 for row-group packing)
        for c in range(NT):
            nc.scalar.dma_start(
                out=krope2_sb[0:64, ds(c * TL, TL)], in_=gout_kv[c, ds(KV_RANK, D_ROPE), :]
            )
            nc.scalar.dma_start(
                out=krope2_sb[64:128, ds(c * TL, TL)], in_=gout_kv[c, ds(KV_RANK, D_ROPE), :]
            )

        wa.release()
        w1.release()
        # SBUF-resident intermediates (reuse w1's region)
        kvsb = tc.alloc_tile_pool(name="kvsb", bufs=1)
        kn_sb = kvsb.tile([128, NH, S], bf16, tag="kn")
        v_sb = kvsb.tile([128, S // 128, NH * V_DIM], bf16, tag="v")
        qn_sb = kvsb.tile([128, NH, S], bf16, tag="qn")
        qr2_sb = kvsb.tile([128, NH, S], bf16, tag="qr2")
        wb = tc.alloc_tile_pool(name="wb", bufs=2)

        # ================= phase A1: kv up-projections per chunk =================
        for c in range(NT):
            csl = ds(c * T, T)
            kvg = wb.tile([128, RKV, T], bf16, tag="kvg", bufs=2)
            for j in range(RKV):
                nc.scalar.dma_start(out=kvg[:, j, :], in_=gout_kv[c, ds(j * 128, 128), :])
            for m in range(NH):
                ps = sc_half(st, m)
                for j in range(RKV):
                    nc.tensor.matmul(
                        ps, wkvuk_sb[:, j, ds(m * 128, 128)], kvg[:, j, :],
                        start=(j == 0), stop=(j == RKV - 1),
                    )
                nc.vector.tensor_copy(kn_sb[:, m, csl], ps)
            for s2 in range(T // 128):
                ps = sc_half(st, s2)
                for j in range(RKV):
                    nc.tensor.matmul(
                        ps, kvg[:, j, ds(s2 * 128, 128)], wkvuv_sb[:, j, :],
                        start=(j == 0), stop=(j == RKV - 1),
                    )
                nc.vector.tensor_copy(v_sb[:, c * (T // 128) + s2, :], ps)

        # ================= phase A2: q up-projections per chunk =================
        for c in range(NT):
            csl = ds(c * T, T)
            qlg = wb.tile([128, RQ, T], bf16, tag="qlg", bufs=2)
            for m in range(RQ):
                src = gout_q1 if m < RQ // 2 else gout_q2
                mm = m if m < RQ // 2 else m - RQ // 2
                nc.scalar.dma_start(out=qlg[:, m, :], in_=src[c, ds(mm * 128, 128), :])
            for m in range(NH):
                ps = sc_half(st, m)
                for r in range(RQ):
                    nc.tensor.matmul(
                        ps, wqu_sb[:, r, ds(m * 128, 128)], qlg[:, r, :],
                        start=(r == 0), stop=(r == RQ - 1),
                    )
                nc.vector.tensor_copy(qn_sb[:, m, csl], ps)
            sct = sc_ps.tile([128, 1024], f32, tag="sc", name="sct_rope")
            ps1, ps2 = sct[:, 0:512], sct[:, 512:1024]
            for r in range(RQ):
                nc.tensor.matmul(
                    ps1, wqu_sb[:, r, ds(NH * D_NOPE, 128)], qlg[:, r, :],
                    start=(r == 0), stop=(r == RQ - 1),
                )
            for r in range(RQ):
                nc.tensor.matmul(
                    ps2, wqu_sb[:, r, ds(NH * D_NOPE + 128, 128)], qlg[:, r, :],
                    start=(r == 0), stop=(r == RQ - 1),
                )
            qa = wb.tile([128, T], f32, tag="qa", bufs=1)
            qb = wb.tile([128, T], f32, tag="qb", bufs=1)
            nc.vector.tensor_mul(qa, ps1, cos_sb[:, csl])
            nc.vector.tensor_mul(qb, ps2, sin_sb[:, csl])
            y1 = wb.tile([128, T], bf16, tag="y1", bufs=2)
            nc.vector.tensor_sub(y1, qa, qb)
            qa2 = wb.tile([128, T], f32, tag="qa", bufs=1)
            qb2 = wb.tile([128, T], f32, tag="qb", bufs=1)
            nc.vector.tensor_mul(qa2, ps2, cos_sb[:, csl])
            nc.vector.tensor_mul(qb2, ps1, sin_sb[:, csl])
            y2 = wb.tile([128, T], bf16, tag="y2", bufs=2)
            nc.vector.tensor_add(y2, qa2, qb2)
            # assemble per-head [x1(32); x2(32)] rope layout, duplicated at 64-127
            # (on the sync queue, which is idle by now)
            for h in range(NH):
                nc.sync.dma_start(out=qr2_sb[0:32, h, csl], in_=y1[ds(32 * h, 32), :])
                nc.sync.dma_start(out=qr2_sb[32:64, h, csl], in_=y2[ds(32 * h, 32), :])
                nc.sync.dma_start(out=qr2_sb[64:96, h, csl], in_=y1[ds(32 * h, 32), :])
                nc.sync.dma_start(out=qr2_sb[96:128, h, csl], in_=y2[ds(32 * h, 32), :])

        # ================= phase B + C: attention with interleaved out-proj =====
        wb.release()
        wc = tc.alloc_tile_pool(name="wc", bufs=2)

        norm_pend = []

        def drain_norm(stn):
            h_, qsl_, pv_, den_ = stn
            rec = wc.tile([128, QC], f32, tag="rec", bufs=2)
            nc.vector.reciprocal_approx_fast(out=rec, in_=den_)
            nc.vector.tensor_mul(at_sb[:, h_, qsl_], pv_, rec)

        for qc in range(NQC):
            qsl = ds(qc * QC, QC)
            nkc = 4 * qc + 4
            npair = nkc // 2
            for h in range(NH):
                pv = pv_ps.tile([128, QC], f32, tag="pv")
                den_ps = aux_ps.tile([128, QC], f32, tag="aux", name="den_ps")
                pend = []
                for t in range(npair):
                    kcA, kcB = 2 * t, 2 * t + 1
                    dA, dB = kcA - 4 * qc, kcB - 4 * qc
                    sct = sc_ps.tile([128, 1024], f32, tag="sc", name="sct_b")
                    nc.tensor.matmul(
                        sct[:, 0:512], kn_sb[:, h, ds(kcA * 128, 128)], qn_sb[:, h, qsl],
                        start=True, stop=False,
                    )
                    nc.tensor.matmul(
                        sct[:, 512:1024], kn_sb[:, h, ds(kcB * 128, 128)], qn_sb[:, h, qsl],
                        start=True, stop=False,
                    )
                    # the two K=64 rope matmuls land in disjoint row-groups -> concurrent
                    nc.tensor.matmul(
                        sct[:, 0:512], krope2_sb[0:64, ds(kcA * 128, 128)],
                        qr2_sb[0:64, h, qsl], start=False, stop=(dA < 0),
                    )
                    nc.tensor.matmul(
                        sct[:, 512:1024], krope2_sb[64:128, ds(kcB * 128, 128)],
                        qr2_sb[64:128, h, qsl], start=False, stop=(dB < 0),
                    )
                    if dA >= 0:
                        # causal mask: add -3e4 on invalid entries via identity matmul
                        pi = dA // 2
                        nc.tensor.matmul(
                            sct[:, 0:512], ident_sb[:], mask_sb[:, pi, 0:512],
                            start=False, stop=True,
                        )
                        nc.tensor.matmul(
                            sct[:, 512:1024], ident_sb[:], mask_sb[:, pi, 512:1024],
                            start=False, stop=True,
                        )
                    E = wc.tile([128, 1024], bf16, tag="E", bufs=4)
                    nc.scalar.activation(E, sct, AF.Exp, scale=SCALE)
                    Eh = wc.tile([128, QC], bf16, tag="Eh", bufs=3)
                    nc.vector.tensor_add(Eh, E[:, 0:512], E[:, 512:1024])
                    pend.append((t, E, Eh))
                    if len(pend) > 1:
                        pt, pE, pEh = pend.pop(0)
                        nc.tensor.matmul(
                            pv, v_sb[:, 2 * pt, ds(h * V_DIM, V_DIM)], pE[:, 0:512],
                            start=(pt == 0), stop=False,
                        )
                        nc.tensor.matmul(
                            pv, v_sb[:, 2 * pt + 1, ds(h * V_DIM, V_DIM)], pE[:, 512:1024],
                            start=False, stop=False,
                        )
                        nc.tensor.matmul(
                            den_ps, ones_sb[:], pEh,
                            start=(pt == 0), stop=False,
                        )
                while pend:
                    pt, pE, pEh = pend.pop(0)
                    last = not pend
                    nc.tensor.matmul(
                        pv, v_sb[:, 2 * pt, ds(h * V_DIM, V_DIM)], pE[:, 0:512],
                        start=(pt == 0), stop=False,
                    )
                    nc.tensor.matmul(
                        pv, v_sb[:, 2 * pt + 1, ds(h * V_DIM, V_DIM)], pE[:, 512:1024],
                        start=False, stop=last,
                    )
                    nc.tensor.matmul(
                        den_ps, ones_sb[:], pEh,
                        start=(pt == 0), stop=last,
                    )
                norm_pend.append((h, qsl, pv, den_ps))
                if len(norm_pend) > 1:
                    drain_norm(norm_pend.pop(0))
            while norm_pend:
                drain_norm(norm_pend.pop(0))
            # ---- out-projection for this qc's 4 token blocks ----
            for t16 in range(qc * 4, qc * 4 + 4):
                for n in range(HID // 512):
                    ps = sc_half(st, n)
                    for f in range(NH):
                        nc.tensor.matmul(
                            ps, at_sb[:, f, ds(t16 * 128, 128)], wout_sb[:, f, ds(n * 512, 512)],
                            start=(f == 0), stop=(f == NH - 1),
                        )
                    o_t = wc.tile([128, 512], f16, tag="ot", bufs=3)
                    nc.vector.tensor_copy(o_t, ps)
                    nc.sync.dma_start(
                        out=out_ap[ds(t16 * 128, 128), ds(n * 512, 512)], in_=o_t
                    )

        wc.release()
        kvsb.release()
        w2.release()

    nc.compile()
    return nc


def get_nc():
    if "nc" not in _CACHE:
        _CACHE["nc"] = build_nc()
    return _CACHE["nc"]


def host_inputs(x, w_q_down, w_q_up, w_kv_down, kv_norm_w, w_kv_up, w_out):
    """Build the 8 per-core input shards (host-side prep, numpy only)."""
    bf = ml_dtypes.bfloat16
    x = np.asarray(x, np.float32)
    inv = 1.0 / THETA ** (np.arange(0, D_ROPE, 2, dtype=np.float64) / D_ROPE)
    ang = np.arange(S, dtype=np.float64)[:, None] * inv[None, :]      # (S, 32)
    cosq = np.ascontiguousarray(np.tile(np.cos(ang).T, (4, 1))).astype(bf)  # (128, S)
    sinq = np.ascontiguousarray(np.tile(np.sin(ang).T, (4, 1))).astype(bf)
    # additive causal masks for the diagonal-band chunks, paired (d, d+1):
    # maskp[r, d//2, 512*(d%2)+j] = 0 if j >= 128*d + r else -3e4
    maskp = np.zeros((128, 2, 1024), np.float32)
    r = np.arange(128)[:, None]
    j = np.arange(512)[None, :]
    for d in range(4):
        maskp[:, d // 2, 512 * (d % 2) : 512 * (d % 2) + 512] = np.where(
            j >= 128 * d + r, 0.0, -30000.0
        )
    maskp = maskp.astype(bf)
    ones128 = np.ones((128, 128), bf)
    ident128 = np.eye(128, dtype=np.float32).astype(bf)
    wkv_eff = np.asarray(w_kv_up, np.float32) * np.asarray(kv_norm_w, np.float32)[:, None]

    xT_bf = [np.ascontiguousarray(x[b].T).astype(bf) for b in range(B)]
    wqd_bf = np.asarray(w_q_down, np.float32).astype(bf)
    wkvd_bf = np.asarray(w_kv_down, np.float32).astype(bf)
    wqu_f = np.asarray(w_q_up, np.float32)
    wout_f = np.asarray(w_out, np.float32)

    in_maps = []
    for ci in range(NCORES):
        b, hg = divmod(ci, 4)
        heads = list(range(NH * hg, NH * hg + NH))
        qu_cols = (
            [h * HEAD_DIM + j for h in heads for j in range(D_NOPE)]
            + [h * HEAD_DIM + D_NOPE + j for h in heads for j in range(32)]
            + [h * HEAD_DIM + D_NOPE + 32 + j for h in heads for j in range(32)]
        )
        kn_cols = [h * (D_NOPE + V_DIM) + j for h in heads for j in range(D_NOPE)]
        v_cols = [h * (D_NOPE + V_DIM) + D_NOPE + j for h in heads for j in range(V_DIM)]
        in_maps.append(
            {
                "x": np.ascontiguousarray(xT_bf[b][:, 512 * hg : 512 * (hg + 1)]),
                "cosl": np.ascontiguousarray(cosq[:, 512 * hg : 512 * (hg + 1)]),
                "sinl": np.ascontiguousarray(sinq[:, 512 * hg : 512 * (hg + 1)]),
                "wqd": wqd_bf,
                "wqu": np.ascontiguousarray(wqu_f[:, qu_cols]).astype(bf),
                "wkvd": wkvd_bf,
                "wkvuk": np.ascontiguousarray(wkv_eff[:, kn_cols]).astype(bf),
                "wkvuv": np.ascontiguousarray(wkv_eff[:, v_cols]).astype(bf),
                "wout": np.ascontiguousarray(
                    wout_f[NH * V_DIM * hg : NH * V_DIM * (hg + 1), :]
                ).astype(bf),
                "cosq": cosq,
                "sinq": sinq,
                "maskp": maskp,
                "ones128": ones128,
                "ident128": ident128,
            }
        )
    return in_maps


def run(inputs, trace=False, trace_cores=None):
    from concourse.bass_utils import run_bass_kernel_spmd

    nc = get_nc()
    in_maps = host_inputs(**inputs)
    res = run_bass_kernel_spmd(
        nc,
        in_maps,
        core_ids=list(range(NCORES)),
        trace=trace,
        trace_cores=trace_cores,
    )
    out = np.zeros((B, S, HID), np.float32)
    for ci in range(NCORES):
        out[ci // 4] += res.results[ci]["out"].astype(np.float32)
    return out, res


def kernel(**inputs):
    out, _ = run(inputs, trace=False)
    return out


# revision 22
# speedup vs baseline: 1.2547x; 1.0318x over previous
"""MLA attention kernel for Trainium2 — 8-core tensor-parallel (self-contained).

Sharding: data-parallel over batch (2) x tensor-parallel over head groups
(4 groups of 4 heads) = 8 cores, SPMD (one NEFF, per-core input shards).
Core ci: batch ci//4, heads [4*(ci%4), 4*(ci%4)+4).

v2 layout: all intermediates (k_nope / v / q_nope / q_rope / attention out)
stay resident in SBUF between phases; only the latent gathers bounce through
DRAM (collectives require it).  Softmax denominators and the rmsnorm scale
are broadcast via an all-ones [128,128] stationary matmul so the reciprocal
runs on 128 DVE lanes.  Score PSUM tiles are [128,1024] (2 banks) so one
ACT exp instruction covers two key chunks; the K=64 rope score matmuls for
the two chunks run concurrently in disjoint PE row-groups (partitions 0-63
vs 64-127).  The out-projection is interleaved into the attention loop per
query chunk, output written fp16.
"""

import math

import numpy as np
import ml_dtypes

# ---- problem constants (from the reference model) ----
B, S, HID = 2, 2048, 2048
H, D_NOPE, D_ROPE, V_DIM = 16, 128, 64, 128
KV_RANK, Q_RANK = 512, 1536
HEAD_DIM = D_NOPE + D_ROPE
THETA, EPS = 10000.0, 1e-6
NCORES = 8
NH = 4                    # heads per core
T = 512                   # token chunk
NT = S // T
QC = 512                  # attention query chunk
NQC = S // QC
KH = HID // 128           # 16 k-chunks over HID
RQ = Q_RANK // 128        # 12 chunks over q rank
RKV = KV_RANK // 128      # 4 chunks over kv rank
SCALE = 1.0 / math.sqrt(HEAD_DIM)

_CACHE = {}


def build_nc():
    """Build the Bass/Tile program (one NeuronCore, run SPMD on 8)."""
    from contextlib import ExitStack

    import concourse.mybir as mybir
    import concourse.tile as tile
    from concourse import bacc
    from concourse.bass import ds

    dt = mybir.dt
    AF = mybir.ActivationFunctionType
    bf16 = dt.bfloat16
    f32 = dt.float32
    f16 = dt.float16

    nc = bacc.Bacc(
        "TRN2",
        target_bir_lowering=False,
        debug=False,
        enable_asserts=False,
        num_devices=NCORES,
    )

    # ---- I/O ----
    x_ap = nc.dram_tensor("x", [HID, S // 4], bf16, kind="ExternalInput").ap()
    wqd_ap = nc.dram_tensor("wqd", [HID, Q_RANK], bf16, kind="ExternalInput").ap()
    wqu_ap = nc.dram_tensor("wqu", [Q_RANK, NH * HEAD_DIM], bf16, kind="ExternalInput").ap()
    wkvd_ap = nc.dram_tensor("wkvd", [HID, KV_RANK + D_ROPE], bf16, kind="ExternalInput").ap()
    wkvuk_ap = nc.dram_tensor("wkvuk", [KV_RANK, NH * D_NOPE], bf16, kind="ExternalInput").ap()
    wkvuv_ap = nc.dram_tensor("wkvuv", [KV_RANK, NH * V_DIM], bf16, kind="ExternalInput").ap()
    wout_ap = nc.dram_tensor("wout", [NH * V_DIM, HID], bf16, kind="ExternalInput").ap()
    cos_ap = nc.dram_tensor("cosq", [128, S], bf16, kind="ExternalInput").ap()
    sin_ap = nc.dram_tensor("sinq", [128, S], bf16, kind="ExternalInput").ap()
    mask_ap = nc.dram_tensor("maskp", [128, 2, 1024], bf16, kind="ExternalInput").ap()
    ones_ap = nc.dram_tensor("ones128", [128, 128], bf16, kind="ExternalInput").ap()
    ident_ap = nc.dram_tensor("ident128", [128, 128], bf16, kind="ExternalInput").ap()
    cosl_ap = nc.dram_tensor("cosl", [128, S // 4], bf16, kind="ExternalInput").ap()
    sinl_ap = nc.dram_tensor("sinl", [128, S // 4], bf16, kind="ExternalInput").ap()
    out_ap = nc.dram_tensor("out", [S, HID], f16, kind="ExternalOutput").ap()

    with tile.TileContext(nc) as tc, ExitStack() as ctx:
        # ---- PSUM pools: 2x[128,1024] + 2x[128,512] + 2x[128,512] = 8 banks
        sc_ps = ctx.enter_context(tc.tile_pool(name="sc_ps", bufs=2, space="PSUM"))
        pv_ps = ctx.enter_context(tc.tile_pool(name="pv_ps", bufs=2, space="PSUM"))
        aux_ps = ctx.enter_context(tc.tile_pool(name="aux_ps", bufs=2, space="PSUM"))

        def sc_half(state, idx):
            # rotate [128,1024] sc tiles, handing out 512-wide halves
            if idx % 2 == 0:
                state["t"] = sc_ps.tile([128, 1024], f32, tag="sc", name="sct")
            return state["t"][:, ds((idx % 2) * 512, 512)]

        const = ctx.enter_context(tc.tile_pool(name="const", bufs=1))
        woutp = ctx.enter_context(tc.tile_pool(name="woutp", bufs=1))
        dram = ctx.enter_context(tc.tile_pool(name="dram", bufs=1, space="DRAM"))

        TL = S // 4  # local token quarter

        # ---- up-projection weights (needed in A1/A2; allocated below w1 so the
        # pool stack stays LIFO: w2 outlives w1)
        w2 = tc.alloc_tile_pool(name="w2", bufs=1)
        wkvuk_sb = w2.tile([128, RKV, NH * D_NOPE], bf16, tag="wkvuk")
        wkvuv_sb = w2.tile([128, RKV, NH * V_DIM], bf16, tag="wkvuv")
        wqu_sb = w2.tile([128, RQ, NH * HEAD_DIM], bf16, tag="wqu")

        # ---- phase-A0 weights + x (released after A0; space reused for kv/q SBUF stores)
        w1 = tc.alloc_tile_pool(name="w1", bufs=1)
        xt = w1.tile([128, KH, TL], bf16, tag="xt")
        wkvd_sb = w1.tile([128, KH, KV_RANK + D_ROPE], bf16, tag="wkvd")
        wqd_sb = w1.tile([128, KH, Q_RANK], bf16, tag="wqd")
        # priority DMA order, split across the sync queue and the gpsimd
        # (software-DGE) queue.  The scalar queue is kept clear: DMA trigger
        # instructions occupy the issuing engine for the whole transfer, and
        # the scalar engine must run the rmsnorm ACT ops + latency-critical
        # packs that gate the collectives.
        def ld(i, **kw):
            (nc.sync if i % 2 == 0 else nc.gpsimd).dma_start(**kw)

        for k in range(KH):
            ld(k, out=wkvd_sb[:, k, :], in_=wkvd_ap[ds(k * 128, 128), :])
            ld(k + 1, out=xt[:, k, :], in_=x_ap[ds(k * 128, 128), :])
        for k in range(KH):
            ld(k, out=wqd_sb[:, k, :], in_=wqd_ap[ds(k * 128, 128), :])
        for j in range(RKV):
            ld(j, out=wkvuk_sb[:, j, :], in_=wkvuk_ap[ds(j * 128, 128), :])
            ld(j + 1, out=wkvuv_sb[:, j, :], in_=wkvuv_ap[ds(j * 128, 128), :])

        # ---- resident constants
        cos_sb = const.tile([128, S], bf16, name="cos_sb")
        ld(0, out=cos_sb[:], in_=cos_ap[:])
        sin_sb = const.tile([128, S], bf16, name="sin_sb")
        ld(1, out=sin_sb[:], in_=sin_ap[:])
        cosl_sb = const.tile([128, TL], bf16, name="cosl_sb")
        ld(0, out=cosl_sb[:], in_=cosl_ap[:])
        sinl_sb = const.tile([128, TL], bf16, name="sinl_sb")
        ld(1, out=sinl_sb[:], in_=sinl_ap[:])
        mask_sb = const.tile([128, 2, 1024], bf16, name="mask_sb")
        ld(0, out=mask_sb[:], in_=mask_ap[:])
        ones_sb = const.tile([128, 128], bf16, name="ones_sb")
        ld(1, out=ones_sb[:], in_=ones_ap[:])
        ident_sb = const.tile([128, 128], bf16, name="ident_sb")
        ld(0, out=ident_sb[:], in_=ident_ap[:])
        for r in range(RQ):
            ld(r, out=wqu_sb[:, r, :], in_=wqu_ap[ds(r * 128, 128), :])
        wout_sb = woutp.tile([128, NH, HID], bf16, tag="wout")
        for f in range(NH):
            ld(f, out=wout_sb[:, f, :], in_=wout_ap[ds(f * 128, 128), :])

        krope2_sb = const.tile([128, S], bf16, name="krope2_sb")
        at_sb = const.tile([128, NH, S], bf16, name="at_sb")
        eps_sb = const.tile([128, 1], f32, name="eps_sb")
        nc.gpsimd.memset(eps_sb[:], EPS)

        # DRAM bounce buffers for the latent gathers
        gin_kv = dram.tile([KV_RANK + D_ROPE, TL], bf16, name="gin_kv")
        gout_kv = dram.tile([4, KV_RANK + D_ROPE, TL], bf16, name="gout_kv")
        gin_q = dram.tile([Q_RANK, TL], bf16, name="gin_q")
        # three rank-group gathers so transfers overlap with A1/A2 compute
        gout_q = [
            dram.tile([4, Q_RANK // 3, TL], bf16, name=f"gout_q{g}") for g in range(3)
        ]
        GROUPS = [[0, 1, 2, 3], [4, 5, 6, 7]]

        # ================= phase A0: local down-projections =================
        # k-outer loops: each weight chunk k is consumed as its DMA lands, so
        # the PE never sits idle waiting for the full weight matrix.
        wa = tc.alloc_tile_pool(name="wa", bufs=2)
        st = {}
        kvc_bf = wa.tile([128, RKV, TL], bf16, tag="kvc", bufs=1)
        sq_bf = wa.tile([128, RKV, TL], bf16, tag="sq", bufs=1)
        ms_ps = aux_ps.tile([128, TL], f32, tag="aux", name="ms_ps")
        krp_ps = aux_ps.tile([128, TL], f32, tag="aux", name="krp_ps")
        kv_ps = [sc_half(st, j) for j in range(RKV)]
        for k in range(KH):
            for j in range(RKV):
                nc.tensor.matmul(
                    kv_ps[j], wkvd_sb[:, k, ds(j * 128, 128)], xt[:, k, :],
                    start=(k == 0), stop=(k == KH - 1),
                )
            nc.tensor.matmul(
                krp_ps[0:64, :], wkvd_sb[:, k, ds(KV_RANK, D_ROPE)], xt[:, k, :],
                start=(k == 0), stop=(k == KH - 1),
            )
        for j in range(RKV):
            nc.scalar.activation(sq_bf[:, j, :], kv_ps[j], AF.Square)
            nc.vector.tensor_copy(kvc_bf[:, j, :], kv_ps[j])
        for j in range(RKV):
            nc.tensor.matmul(
                ms_ps, ones_sb[:], sq_bf[:, j, :],
                start=(j == 0), stop=(j == RKV - 1),
            )
        # rinv = 1/sqrt(ms/512 + eps), broadcast across partitions by the ones-matmul
        srt = wa.tile([128, TL], f32, tag="srt", bufs=1)
        nc.scalar.activation(srt, ms_ps, AF.Sqrt, bias=eps_sb[:], scale=1.0 / KV_RANK)
        rinv = wa.tile([128, TL], f32, tag="rinv", bufs=1)
        nc.vector.reciprocal_approx_fast(out=rinv, in_=srt)
        kvcn = wa.tile([128, RKV, TL], bf16, tag="kvcn", bufs=1)
        for j in range(RKV):
            nc.vector.tensor_mul(kvcn[:, j, :], kvc_bf[:, j, :], rinv)
            nc.scalar.dma_start(out=gin_kv[ds(j * 128, 128), :], in_=kvcn[:, j, :])
        # k rope rotate (local quarter, local cos/sin)
        kr_raw = wa.tile([64, TL], f32, tag="kr_raw", bufs=1)
        nc.vector.tensor_copy(kr_raw, krp_ps[0:64, :])
        kr_sh = wa.tile([64, TL], f32, tag="kr_sh", bufs=1)
        nc.scalar.dma_start(out=kr_sh[0:32, :], in_=kr_raw[32:64, :])
        nc.scalar.dma_start(out=kr_sh[32:64, :], in_=kr_raw[0:32, :])
        kt1 = wa.tile([64, TL], f32, tag="kt1", bufs=1)
        kt2 = wa.tile([64, TL], f32, tag="kt2", bufs=1)
        nc.vector.tensor_mul(kt1, kr_raw, cosl_sb[0:64, :])
        nc.vector.tensor_mul(kt2, kr_sh, sinl_sb[0:64, :])
        krl = wa.tile([64, TL], bf16, tag="krl", bufs=1)
        nc.vector.tensor_sub(krl[0:32, :], kt1[0:32, :], kt2[0:32, :])
        nc.vector.tensor_add(krl[32:64, :], kt1[32:64, :], kt2[32:64, :])
        nc.scalar.dma_start(out=gin_kv[ds(KV_RANK, D_ROPE), :], in_=krl[:])
        nc.gpsimd.collective_compute(
            "AllGather", mybir.AluOpType.bypass, replica_groups=GROUPS,
            ins=[gin_kv.opt()], outs=[gout_kv.opt()],
        )

        # ---- q down (local quarter): k-outer over m-groups of 4, gathering
        # each rank-group as soon as it completes ----
        qlat = wa.tile([128, RQ, TL], bf16, tag="qlat", bufs=1)
        for g in range(3):
            qps = [sc_half(st, m) for m in range(4)]
            for k in range(KH):
                for m in range(4):
                    nc.tensor.matmul(
                        qps[m], wqd_sb[:, k, ds((4 * g + m) * 128, 128)], xt[:, k, :],
                        start=(k == 0), stop=(k == KH - 1),
                    )
            for m in range(4):
                gm = 4 * g + m
                nc.vector.tensor_copy(qlat[:, gm, :], qps[m])
                nc.scalar.dma_start(out=gin_q[ds(gm * 128, 128), :], in_=qlat[:, gm, :])
            nc.gpsimd.collective_compute(
                "AllGather", mybir.AluOpType.bypass, replica_groups=GROUPS,
                ins=[gin_q[ds(g * 512, 512), :].opt()], outs=[gout_q[g].opt()],
            )

        # krope full (duplicated on partitions 64-127 for row-group packing)
        for c in range(NT):
            nc.scalar.dma_start(
                out=krope2_sb[0:64, ds(c * TL, TL)], in_=gout_kv[c, ds(KV_RANK, D_ROPE), :]
            )
            nc.scalar.dma_start(
                out=krope2_sb[64:128, ds(c * TL, TL)], in_=gout_kv[c, ds(KV_RANK, D_ROPE), :]
            )

        wa.release()
        w1.release()
        # SBUF-resident intermediates (reuse w1's region)
        kvsb = tc.alloc_tile_pool(name="kvsb", bufs=1)
        kn_sb = kvsb.tile([128, NH, S], bf16, tag="kn")
        v_sb = kvsb.tile([128, S // 128, NH * V_DIM], bf16, tag="v")
        qn_sb = kvsb.tile([128, NH, S], bf16, tag="qn")
        qr2_sb = kvsb.tile([128, NH, S], bf16, tag="qr2")
        wb = tc.alloc_tile_pool(name="wb", bufs=2)

        # ================= phase A1: kv up-projections per chunk =================
        for c in range(NT):
            csl = ds(c * T, T)
            kvg = wb.tile([128, RKV, T], bf16, tag="kvg", bufs=2)
            for j in range(RKV):
                nc.scalar.dma_start(out=kvg[:, j, :], in_=gout_kv[c, ds(j * 128, 128), :])
            for m in range(NH):
                ps = sc_half(st, m)
                for j in range(RKV):
                    nc.tensor.matmul(
                        ps, wkvuk_sb[:, j, ds(m * 128, 128)], kvg[:, j, :],
                        start=(j == 0), stop=(j == RKV - 1),
                    )
                nc.vector.tensor_copy(kn_sb[:, m, csl], ps)
            for s2 in range(T // 128):
                ps = sc_half(st, s2)
                for j in range(RKV):
                    nc.tensor.matmul(
                        ps, kvg[:, j, ds(s2 * 128, 128)], wkvuv_sb[:, j, :],
                        start=(j == 0), stop=(j == RKV - 1),
                    )
                nc.vector.tensor_copy(v_sb[:, c * (T // 128) + s2, :], ps)

        # ================= phase A2: q up-projections per chunk =================
        # r-grouped accumulation: rank-group g's matmuls only need the g-th
        # q gather, so chunk c starts on gather 0 while gathers 1/2 transfer.
        for c in range(NT):
            csl = ds(c * T, T)
            qlg = wb.tile([128, RQ, T], bf16, tag="qlg", bufs=2)
            qn_ps = [sc_half(st, m) for m in range(NH)]
            rp = pv_ps.tile([128, T], f32, tag="pv", name="rp1")
            rp2 = pv_ps.tile([128, T], f32, tag="pv", name="rp2")
            ps1, ps2 = rp, rp2
            for g in range(3):
                for m in range(4):
                    gm = 4 * g + m
                    nc.scalar.dma_start(
                        out=qlg[:, gm, :], in_=gout_q[g][c, ds(m * 128, 128), :]
                    )
            for r in range(RQ):
                for m in range(NH):
                    nc.tensor.matmul(
                        qn_ps[m], wqu_sb[:, r, ds(m * 128, 128)], qlg[:, r, :],
                        start=(r == 0), stop=(r == RQ - 1),
                    )
                nc.tensor.matmul(
                    ps1, wqu_sb[:, r, ds(NH * D_NOPE, 128)], qlg[:, r, :],
                    start=(r == 0), stop=(r == RQ - 1),
                )
                nc.tensor.matmul(
                    ps2, wqu_sb[:, r, ds(NH * D_NOPE + 128, 128)], qlg[:, r, :],
                    start=(r == 0), stop=(r == RQ - 1),
                )
            for m in range(NH):
                nc.vector.tensor_copy(qn_sb[:, m, csl], qn_ps[m])
            qa = wb.tile([128, T], f32, tag="qa", bufs=1)
            qb = wb.tile([128, T], f32, tag="qb", bufs=1)
            nc.vector.tensor_mul(qa, ps1, cos_sb[:, csl])
            nc.vector.tensor_mul(qb, ps2, sin_sb[:, csl])
            y1 = wb.tile([128, T], bf16, tag="y1", bufs=2)
            nc.vector.tensor_sub(y1, qa, qb)
            qa2 = wb.tile([128, T], f32, tag="qa", bufs=1)
            qb2 = wb.tile([128, T], f32, tag="qb", bufs=1)
            nc.vector.tensor_mul(qa2, ps2, cos_sb[:, csl])
            nc.vector.tensor_mul(qb2, ps1, sin_sb[:, csl])
            y2 = wb.tile([128, T], bf16, tag="y2", bufs=2)
            nc.vector.tensor_add(y2, qa2, qb2)
            # assemble per-head [x1(32); x2(32)] rope layout, duplicated at 64-127
            # (on the sync queue, which is idle by now)
            for h in range(NH):
                nc.sync.dma_start(out=qr2_sb[0:32, h, csl], in_=y1[ds(32 * h, 32), :])
                nc.sync.dma_start(out=qr2_sb[32:64, h, csl], in_=y2[ds(32 * h, 32), :])
                nc.sync.dma_start(out=qr2_sb[64:96, h, csl], in_=y1[ds(32 * h, 32), :])
                nc.sync.dma_start(out=qr2_sb[96:128, h, csl], in_=y2[ds(32 * h, 32), :])

        # ================= phase B + C: attention with interleaved out-proj =====
        wb.release()
        wc = tc.alloc_tile_pool(name="wc", bufs=2)

        norm_pend = []

        def drain_norm(stn):
            h_, qsl_, pv_, den_ = stn
            rec = wc.tile([128, QC], f32, tag="rec", bufs=2)
            nc.vector.reciprocal_approx_fast(out=rec, in_=den_)
            nc.vector.tensor_mul(at_sb[:, h_, qsl_], pv_, rec)

        for qc in range(NQC):
            qsl = ds(qc * QC, QC)
            nkc = 4 * qc + 4
            npair = nkc // 2
            for h in range(NH):
                pv = pv_ps.tile([128, QC], f32, tag="pv")
                den_ps = aux_ps.tile([128, QC], f32, tag="aux", name="den_ps")
                pend = []
                for t in range(npair):
                    kcA, kcB = 2 * t, 2 * t + 1
                    dA, dB = kcA - 4 * qc, kcB - 4 * qc
                    sct = sc_ps.tile([128, 1024], f32, tag="sc", name="sct_b")
                    nc.tensor.matmul(
                        sct[:, 0:512], kn_sb[:, h, ds(kcA * 128, 128)], qn_sb[:, h, qsl],
                        start=True, stop=False,
                    )
                    nc.tensor.matmul(
                        sct[:, 512:1024], kn_sb[:, h, ds(kcB * 128, 128)], qn_sb[:, h, qsl],
                        start=True, stop=False,
                    )
                    # the two K=64 rope matmuls land in disjoint row-groups -> concurrent
                    nc.tensor.matmul(
                        sct[:, 0:512], krope2_sb[0:64, ds(kcA * 128, 128)],
                        qr2_sb[0:64, h, qsl], start=False, stop=(dA < 0),
                    )
                    nc.tensor.matmul(
                        sct[:, 512:1024], krope2_sb[64:128, ds(kcB * 128, 128)],
                        qr2_sb[64:128, h, qsl], start=False, stop=(dB < 0),
                    )
                    if dA >= 0:
                        # causal mask: add -3e4 on invalid entries via identity matmul
                        pi = dA // 2
                        nc.tensor.matmul(
                            sct[:, 0:512], ident_sb[:], mask_sb[:, pi, 0:512],
                            start=False, stop=True,
                        )
                        nc.tensor.matmul(
                            sct[:, 512:1024], ident_sb[:], mask_sb[:, pi, 512:1024],
                            start=False, stop=True,
                        )
                    E = wc.tile([128, 1024], bf16, tag="E", bufs=6)
                    nc.scalar.activation(E, sct, AF.Exp, scale=SCALE)
                    Eh = wc.tile([128, QC], bf16, tag="Eh", bufs=4)
                    nc.vector.tensor_add(Eh, E[:, 0:512], E[:, 512:1024])
                    pend.append((t, E, Eh))
                    if len(pend) > (2 if npair >= 3 else 1):
                        pt, pE, pEh = pend.pop(0)
                        nc.tensor.matmul(
                            pv, v_sb[:, 2 * pt, ds(h * V_DIM, V_DIM)], pE[:, 0:512],
                            start=(pt == 0), stop=False,
                        )
                        nc.tensor.matmul(
                            pv, v_sb[:, 2 * pt + 1, ds(h * V_DIM, V_DIM)], pE[:, 512:1024],
                            start=False, stop=False,
                        )
                        nc.tensor.matmul(
                            den_ps, ones_sb[:], pEh,
                            start=(pt == 0), stop=False,
                        )
                while pend:
                    pt, pE, pEh = pend.pop(0)
                    last = not pend
                    nc.tensor.matmul(
                        pv, v_sb[:, 2 * pt, ds(h * V_DIM, V_DIM)], pE[:, 0:512],
                        start=(pt == 0), stop=False,
                    )
                    nc.tensor.matmul(
                        pv, v_sb[:, 2 * pt + 1, ds(h * V_DIM, V_DIM)], pE[:, 512:1024],
                        start=False, stop=last,
                    )
                    nc.tensor.matmul(
                        den_ps, ones_sb[:], pEh,
                        start=(pt == 0), stop=last,
                    )
                norm_pend.append((h, qsl, pv, den_ps))
                if len(norm_pend) > 1:
                    drain_norm(norm_pend.pop(0))
            while norm_pend:
                drain_norm(norm_pend.pop(0))
            # ---- out-projection for this qc's 4 token blocks ----
            for t16 in range(qc * 4, qc * 4 + 4):
                for n in range(HID // 512):
                    ps = sc_half(st, n)
                    for f in range(NH):
                        nc.tensor.matmul(
                            ps, at_sb[:, f, ds(t16 * 128, 128)], wout_sb[:, f, ds(n * 512, 512)],
                            start=(f == 0), stop=(f == NH - 1),
                        )
                    o_t = wc.tile([128, 512], f16, tag="ot", bufs=3)
                    nc.vector.tensor_copy(o_t, ps)
                    nc.sync.dma_start(
                        out=out_ap[ds(t16 * 128, 128), ds(n * 512, 512)], in_=o_t
                    )

        wc.release()
        kvsb.release()
        w2.release()

    nc.compile()
    return nc


def get_nc():
    if "nc" not in _CACHE:
        _CACHE["nc"] = build_nc()
    return _CACHE["nc"]


def host_inputs(x, w_q_down, w_q_up, w_kv_down, kv_norm_w, w_kv_up, w_out):
    """Build the 8 per-core input shards (host-side prep, numpy only)."""
    bf = ml_dtypes.bfloat16
    x = np.asarray(x, np.float32)
    inv = 1.0 / THETA ** (np.arange(0, D_ROPE, 2, dtype=np.float64) / D_ROPE)
    ang = np.arange(S, dtype=np.float64)[:, None] * inv[None, :]      # (S, 32)
    cosq = np.ascontiguousarray(np.tile(np.cos(ang).T, (4, 1))).astype(bf)  # (128, S)
    sinq = np.ascontiguousarray(np.tile(np.sin(ang).T, (4, 1))).astype(bf)
    # additive causal masks for the diagonal-band chunks, paired (d, d+1):
    # maskp[r, d//2, 512*(d%2)+j] = 0 if j >= 128*d + r else -3e4
    maskp = np.zeros((128, 2, 1024), np.float32)
    r = np.arange(128)[:, None]
    j = np.arange(512)[None, :]
    for d in range(4):
        maskp[:, d // 2, 512 * (d % 2) : 512 * (d % 2) + 512] = np.where(
            j >= 128 * d + r, 0.0, -30000.0
        )
    maskp = maskp.astype(bf)
    ones128 = np.ones((128, 128), bf)
    ident128 = np.eye(128, dtype=np.float32).astype(bf)
    wkv_eff = np.asarray(w_kv_up, np.float32) * np.asarray(kv_norm_w, np.float32)[:, None]

    xT_bf = [np.ascontiguousarray(x[b].T).astype(bf) for b in range(B)]
    wqd_bf = np.asarray(w_q_down, np.float32).astype(bf)
    wkvd_bf = np.asarray(w_kv_down, np.float32).astype(bf)
    wqu_f = np.asarray(w_q_up, np.float32)
    wout_f = np.asarray(w_out, np.float32)

    in_maps = []
    for ci in range(NCORES):
        b, hg = divmod(ci, 4)
        heads = list(range(NH * hg, NH * hg + NH))
        qu_cols = (
            [h * HEAD_DIM + j for h in heads for j in range(D_NOPE)]
            + [h * HEAD_DIM + D_NOPE + j for h in heads for j in range(32)]
            + [h * HEAD_DIM + D_NOPE + 32 + j for h in heads for j in range(32)]
        )
        kn_cols = [h * (D_NOPE + V_DIM) + j for h in heads for j in range(D_NOPE)]
        v_cols = [h * (D_NOPE + V_DIM) + D_NOPE + j for h in heads for j in range(V_DIM)]
        in_maps.append(
            {
                "x": np.ascontiguousarray(xT_bf[b][:, 512 * hg : 512 * (hg + 1)]),
                "cosl": np.ascontiguousarray(cosq[:, 512 * hg : 512 * (hg + 1)]),
                "sinl": np.ascontiguousarray(sinq[:, 512 * hg : 512 * (hg + 1)]),
                "wqd": wqd_bf,
                "wqu": np.ascontiguousarray(wqu_f[:, qu_cols]).astype(bf),
                "wkvd": wkvd_bf,
                "wkvuk": np.ascontiguousarray(wkv_eff[:, kn_cols]).astype(bf),
                "wkvuv": np.ascontiguousarray(wkv_eff[:, v_cols]).astype(bf),
                "wout": np.ascontiguousarray(
                    wout_f[NH * V_DIM * hg : NH * V_DIM * (hg + 1), :]
                ).astype(bf),
                "cosq": cosq,
                "sinq": sinq,
                "maskp": maskp,
                "ones128": ones128,
                "ident128": ident128,
            }
        )
    return in_maps


def run(inputs, trace=False, trace_cores=None):
    from concourse.bass_utils import run_bass_kernel_spmd

    nc = get_nc()
    in_maps = host_inputs(**inputs)
    res = run_bass_kernel_spmd(
        nc,
        in_maps,
        core_ids=list(range(NCORES)),
        trace=trace,
        trace_cores=trace_cores,
    )
    out = np.zeros((B, S, HID), np.float32)
    for ci in range(NCORES):
        out[ci // 4] += res.results[ci]["out"].astype(np.float32)
    return out, res


def kernel(**inputs):
    out, _ = run(inputs, trace=False)
    return out


# revision 23
# speedup vs baseline: 1.3586x; 1.0829x over previous
"""MLA attention kernel for Trainium2 — 8-core tensor-parallel (self-contained).

Sharding: data-parallel over batch (2) x tensor-parallel over head groups
(4 groups of 4 heads) = 8 cores, SPMD (one NEFF, per-core input shards).
Core ci: batch ci//4, heads [4*(ci%4), 4*(ci%4)+4).

Layout highlights:
  - every weight is host-pre-reshaped into its SBUF-resident partition-major
    layout so it loads as ONE large DMA (amortizes the ~2us DMA fixed cost);
    wqd is blocked by rank-group so q-down consumes blocks as they land
  - all intermediates (k_nope / v / q_nope / q_rope / attention out) stay in
    SBUF between phases; only the latent gathers bounce through DRAM
  - collective buffers are partition-major, so packs/readbacks are single
    transfers; the k-rope row-duplication needed for PE row-group packing is
    baked into the gathered block
  - softmax denominators use an all-ones [128,128] stationary matmul per
    chunk-pair (broadcast into all 128 partitions -> reciprocal runs on all
    DVE lanes); the causal mask is ADDED to score PSUM via an identity
    matmul before exp, keeping the DVE off the critical path
  - score PSUM tiles are [128,1024] (2 banks): one ACT exp instruction
    covers two key chunks; the two K=64 rope matmuls of a chunk pair run
    concurrently in disjoint PE row-groups (partitions 0-63 / 64-127)
  - out-projection is interleaved into the attention loop per query chunk,
    output written fp16
"""

import math

import numpy as np
import ml_dtypes

# ---- problem constants (from the reference model) ----
B, S, HID = 2, 2048, 2048
H, D_NOPE, D_ROPE, V_DIM = 16, 128, 64, 128
KV_RANK, Q_RANK = 512, 1536
HEAD_DIM = D_NOPE + D_ROPE
THETA, EPS = 10000.0, 1e-6
NCORES = 8
NH = 4                    # heads per core
T = 512                   # token chunk
NT = S // T
QC = 512                  # attention query chunk
NQC = S // QC
KH = HID // 128           # 16 k-chunks over HID
RQ = Q_RANK // 128        # 12 chunks over q rank
RKV = KV_RANK // 128      # 4 chunks over kv rank
SCALE = 1.0 / math.sqrt(HEAD_DIM)

_CACHE = {}


def build_nc():
    """Build the Bass/Tile program (one NeuronCore, run SPMD on 8)."""
    from contextlib import ExitStack

    import concourse.mybir as mybir
    import concourse.tile as tile
    from concourse import bacc
    from concourse.bass import ds

    dt = mybir.dt
    AF = mybir.ActivationFunctionType
    bf16 = dt.bfloat16
    f32 = dt.float32
    f16 = dt.float16

    nc = bacc.Bacc(
        "TRN2",
        target_bir_lowering=False,
        debug=False,
        enable_asserts=False,
        num_devices=NCORES,
    )

    TL = S // 4  # local token quarter

    # ---- I/O (all partition-major, host pre-reshaped) ----
    x_ap = nc.dram_tensor("x", [128, KH, TL], bf16, kind="ExternalInput").ap()
    wqd_ap = nc.dram_tensor("wqd", [128, 3, KH, 512], bf16, kind="ExternalInput").ap()
    wqu_ap = nc.dram_tensor("wqu", [128, RQ, NH * HEAD_DIM], bf16, kind="ExternalInput").ap()
    wkvd_ap = nc.dram_tensor("wkvd", [128, KH, KV_RANK + D_ROPE], bf16, kind="ExternalInput").ap()
    wkvuk_ap = nc.dram_tensor("wkvuk", [128, RKV, NH * D_NOPE], bf16, kind="ExternalInput").ap()
    wkvuv_ap = nc.dram_tensor("wkvuv", [128, RKV, NH * V_DIM], bf16, kind="ExternalInput").ap()
    wout_ap = nc.dram_tensor("wout", [128, NH, HID], bf16, kind="ExternalInput").ap()
    cos_ap = nc.dram_tensor("cosq", [128, S], bf16, kind="ExternalInput").ap()
    sin_ap = nc.dram_tensor("sinq", [128, S], bf16, kind="ExternalInput").ap()
    mask_ap = nc.dram_tensor("maskp", [128, 2, 1024], bf16, kind="ExternalInput").ap()
    ones_ap = nc.dram_tensor("ones128", [128, 128], bf16, kind="ExternalInput").ap()
    ident_ap = nc.dram_tensor("ident128", [128, 128], bf16, kind="ExternalInput").ap()
    cosl_ap = nc.dram_tensor("cosl", [128, TL], bf16, kind="ExternalInput").ap()
    sinl_ap = nc.dram_tensor("sinl", [128, TL], bf16, kind="ExternalInput").ap()
    out_ap = nc.dram_tensor("out", [S, HID], f16, kind="ExternalOutput").ap()

    with tile.TileContext(nc) as tc, ExitStack() as ctx:
        # ---- PSUM pools: 2x[128,1024] + 2x[128,512] + 2x[128,512] = 8 banks
        sc_ps = ctx.enter_context(tc.tile_pool(name="sc_ps", bufs=2, space="PSUM"))
        pv_ps = ctx.enter_context(tc.tile_pool(name="pv_ps", bufs=2, space="PSUM"))
        aux_ps = ctx.enter_context(tc.tile_pool(name="aux_ps", bufs=2, space="PSUM"))

        def sc_half(state, idx):
            # rotate [128,1024] sc tiles, handing out 512-wide halves
            if idx % 2 == 0:
                state["t"] = sc_ps.tile([128, 1024], f32, tag="sc", name="sct")
            return state["t"][:, ds((idx % 2) * 512, 512)]

        const = ctx.enter_context(tc.tile_pool(name="const", bufs=1))
        woutp = ctx.enter_context(tc.tile_pool(name="woutp", bufs=1))
        dram = ctx.enter_context(tc.tile_pool(name="dram", bufs=1, space="DRAM"))

        # ---- up-projection weights (outlive w1 -> allocated below it)
        w2 = tc.alloc_tile_pool(name="w2", bufs=1)
        wkvuk_sb = w2.tile([128, RKV, NH * D_NOPE], bf16, tag="wkvuk")
        wkvuv_sb = w2.tile([128, RKV, NH * V_DIM], bf16, tag="wkvuv")
        wqu_sb = w2.tile([128, RQ, NH * HEAD_DIM], bf16, tag="wqu")

        # ---- phase-A0 weights + x (released after A0)
        w1 = tc.alloc_tile_pool(name="w1", bufs=1)
        xt = w1.tile([128, KH, TL], bf16, tag="xt")
        wkvd_sb = w1.tile([128, KH, KV_RANK + D_ROPE], bf16, tag="wkvd")
        wqd_sb = w1.tile([128, 3, KH, 512], bf16, tag="wqd")

        # priority-ordered big transfers, all on the sync queue; the scalar
        # queue stays clear for ACT compute + latency-critical packs.
        for half in range(2):
            hk = ds(half * (KH // 2), KH // 2)
            nc.sync.dma_start(out=xt[:, hk, :], in_=x_ap[:, hk, :])
            nc.sync.dma_start(out=wkvd_sb[:, hk, :], in_=wkvd_ap[:, hk, :])
        for g in range(3):
            nc.sync.dma_start(out=wqd_sb[:, g, :, :], in_=wqd_ap[:, g, :, :])
        nc.sync.dma_start(out=wkvuk_sb[:], in_=wkvuk_ap[:])
        nc.sync.dma_start(out=wkvuv_sb[:], in_=wkvuv_ap[:])

        # ---- resident constants
        cos_sb = const.tile([128, S], bf16, name="cos_sb")
        nc.sync.dma_start(out=cos_sb[:], in_=cos_ap[:])
        sin_sb = const.tile([128, S], bf16, name="sin_sb")
        nc.sync.dma_start(out=sin_sb[:], in_=sin_ap[:])
        cosl_sb = const.tile([128, TL], bf16, name="cosl_sb")
        nc.sync.dma_start(out=cosl_sb[:], in_=cosl_ap[:])
        sinl_sb = const.tile([128, TL], bf16, name="sinl_sb")
        nc.sync.dma_start(out=sinl_sb[:], in_=sinl_ap[:])
        mask_sb = const.tile([128, 2, 1024], bf16, name="mask_sb")
        nc.sync.dma_start(out=mask_sb[:], in_=mask_ap[:])
        ones_sb = const.tile([128, 128], bf16, name="ones_sb")
        nc.sync.dma_start(out=ones_sb[:], in_=ones_ap[:])
        ident_sb = const.tile([128, 128], bf16, name="ident_sb")
        nc.sync.dma_start(out=ident_sb[:], in_=ident_ap[:])
        nc.sync.dma_start(out=wqu_sb[:], in_=wqu_ap[:])
        wout_sb = woutp.tile([128, NH, HID], bf16, tag="wout")
        nc.sync.dma_start(out=wout_sb[:], in_=wout_ap[:])

        krope2_sb = const.tile([128, S], bf16, name="krope2_sb")
        at_sb = const.tile([128, NH, S], bf16, name="at_sb")
        eps_sb = const.tile([128, 1], f32, name="eps_sb")
        nc.gpsimd.memset(eps_sb[:], EPS)

        # DRAM bounce buffers for the latent gathers (partition-major; the
        # kv block 4 carries the row-duplicated rotated k-rope)
        gin_kv = dram.tile([128, RKV + 1, TL], bf16, name="gin_kv")
        gout_kv = dram.tile([4, 128, RKV + 1, TL], bf16, name="gout_kv")
        gin_q = [dram.tile([128, 4, TL], bf16, name=f"gin_q{g}") for g in range(3)]
        gout_q = [dram.tile([4, 128, 4, TL], bf16, name=f"gout_q{g}") for g in range(3)]
        GROUPS = [[0, 1, 2, 3], [4, 5, 6, 7]]

        # ================= phase A0: local down-projections =================
        # k-outer: each weight chunk is consumed as soon as its DMA lands.
        wa = tc.alloc_tile_pool(name="wa", bufs=2)
        st = {}
        kvc_bf = wa.tile([128, RKV, TL], bf16, tag="kvc", bufs=1)
        sq_bf = wa.tile([128, RKV, TL], bf16, tag="sq", bufs=1)
        ms_ps = aux_ps.tile([128, TL], f32, tag="aux", name="ms_ps")
        krp_ps = aux_ps.tile([128, TL], f32, tag="aux", name="krp_ps")
        kv_ps = [sc_half(st, j) for j in range(RKV)]
        for k in range(KH):
            for j in range(RKV):
                nc.tensor.matmul(
                    kv_ps[j], wkvd_sb[:, k, ds(j * 128, 128)], xt[:, k, :],
                    start=(k == 0), stop=(k == KH - 1),
                )
            nc.tensor.matmul(
                krp_ps[0:64, :], wkvd_sb[:, k, ds(KV_RANK, D_ROPE)], xt[:, k, :],
                start=(k == 0), stop=(k == KH - 1),
            )
        for j in range(RKV):
            nc.scalar.activation(sq_bf[:, j, :], kv_ps[j], AF.Square)
            nc.vector.tensor_copy(kvc_bf[:, j, :], kv_ps[j])
        for j in range(RKV):
            nc.tensor.matmul(
                ms_ps, ones_sb[:], sq_bf[:, j, :],
                start=(j == 0), stop=(j == RKV - 1),
            )
        # rinv = 1/sqrt(ms/512 + eps), already broadcast across partitions
        srt = wa.tile([128, TL], f32, tag="srt", bufs=1)
        nc.scalar.activation(srt, ms_ps, AF.Sqrt, bias=eps_sb[:], scale=1.0 / KV_RANK)
        rinv = wa.tile([128, TL], f32, tag="rinv", bufs=1)
        nc.vector.reciprocal_approx_fast(out=rinv, in_=srt)
        kvcn = wa.tile([128, RKV, TL], bf16, tag="kvcn", bufs=1)
        for j in range(RKV):
            nc.vector.tensor_mul(kvcn[:, j, :], kvc_bf[:, j, :], rinv)
        nc.scalar.dma_start(out=gin_kv[:, 0:RKV, :], in_=kvcn[:])
        # k rope rotate (local quarter, local cos/sin), duplicated to 64-127
        kr_raw = wa.tile([64, TL], f32, tag="kr_raw", bufs=1)
        nc.vector.tensor_copy(kr_raw, krp_ps[0:64, :])
        kr_sh = wa.tile([64, TL], f32, tag="kr_sh", bufs=1)
        nc.scalar.dma_start(out=kr_sh[0:32, :], in_=kr_raw[32:64, :])
        nc.scalar.dma_start(out=kr_sh[32:64, :], in_=kr_raw[0:32, :])
        kt1 = wa.tile([64, TL], f32, tag="kt1", bufs=1)
        kt2 = wa.tile([64, TL], f32, tag="kt2", bufs=1)
        nc.vector.tensor_mul(kt1, kr_raw, cosl_sb[0:64, :])
        nc.vector.tensor_mul(kt2, kr_sh, sinl_sb[0:64, :])
        krl2 = wa.tile([128, TL], bf16, tag="krl2", bufs=1)
        nc.vector.tensor_sub(krl2[0:32, :], kt1[0:32, :], kt2[0:32, :])
        nc.vector.tensor_add(krl2[32:64, :], kt1[32:64, :], kt2[32:64, :])
        nc.scalar.dma_start(out=krl2[64:128, :], in_=krl2[0:64, :])
        nc.scalar.dma_start(out=gin_kv[:, RKV, :], in_=krl2[:])
        nc.gpsimd.collective_compute(
            "AllGather", mybir.AluOpType.bypass, replica_groups=GROUPS,
            ins=[gin_kv.opt()], outs=[gout_kv.opt()],
        )

        # ---- q down: k-outer over rank-groups of 4, gathering each group
        # as soon as it completes ----
        qlat = wa.tile([128, RQ, TL], bf16, tag="qlat", bufs=1)
        for g in range(3):
            qps = [sc_half(st, m) for m in range(4)]
            for k in range(KH):
                for m in range(4):
                    nc.tensor.matmul(
                        qps[m], wqd_sb[:, g, k, ds(m * 128, 128)], xt[:, k, :],
                        start=(k == 0), stop=(k == KH - 1),
                    )
            for m in range(4):
                nc.vector.tensor_copy(qlat[:, 4 * g + m, :], qps[m])
            nc.scalar.dma_start(out=gin_q[g][:], in_=qlat[:, ds(4 * g, 4), :])
            nc.gpsimd.collective_compute(
                "AllGather", mybir.AluOpType.bypass, replica_groups=GROUPS,
                ins=[gin_q[g].opt()], outs=[gout_q[g].opt()],
            )

        # krope full (block 4 of the kv gather, already row-duplicated)
        for c in range(NT):
            nc.scalar.dma_start(
                out=krope2_sb[:, ds(c * TL, TL)], in_=gout_kv[c, :, RKV, :]
            )

        wa.release()
        w1.release()
        # SBUF-resident intermediates (reuse w1's region)
        kvsb = tc.alloc_tile_pool(name="kvsb", bufs=1)
        kn_sb = kvsb.tile([128, NH, S], bf16, tag="kn")
        v_sb = kvsb.tile([128, S // 128, NH * V_DIM], bf16, tag="v")
        qn_sb = kvsb.tile([128, NH, S], bf16, tag="qn")
        qr2_sb = kvsb.tile([128, NH, S], bf16, tag="qr2")
        wb = tc.alloc_tile_pool(name="wb", bufs=2)

        # ================= phase A1: kv up-projections per chunk =================
        for c in range(NT):
            csl = ds(c * T, T)
            kvg = wb.tile([128, RKV, T], bf16, tag="kvg", bufs=2)
            nc.scalar.dma_start(out=kvg[:], in_=gout_kv[c, :, 0:RKV, :])
            for m in range(NH):
                ps = sc_half(st, m)
                for j in range(RKV):
                    nc.tensor.matmul(
                        ps, wkvuk_sb[:, j, ds(m * 128, 128)], kvg[:, j, :],
                        start=(j == 0), stop=(j == RKV - 1),
                    )
                nc.vector.tensor_copy(kn_sb[:, m, csl], ps)
            for s2 in range(T // 128):
                ps = sc_half(st, s2)
                for j in range(RKV):
                    nc.tensor.matmul(
                        ps, kvg[:, j, ds(s2 * 128, 128)], wkvuv_sb[:, j, :],
                        start=(j == 0), stop=(j == RKV - 1),
                    )
                nc.vector.tensor_copy(v_sb[:, c * (T // 128) + s2, :], ps)

        # ================= phase A2: q up-projections per chunk =================
        # r-grouped: rank-group g's matmuls only need the g-th q gather.
        for c in range(NT):
            csl = ds(c * T, T)
            qlg = wb.tile([128, RQ, T], bf16, tag="qlg", bufs=2)
            qn_ps = [sc_half(st, m) for m in range(NH)]
            ps1 = pv_ps.tile([128, T], f32, tag="pv", name="rp1")
            ps2 = pv_ps.tile([128, T], f32, tag="pv", name="rp2")
            for g in range(3):
                nc.scalar.dma_start(out=qlg[:, ds(4 * g, 4), :], in_=gout_q[g][c])
            for r in range(RQ):
                for m in range(NH):
                    nc.tensor.matmul(
                        qn_ps[m], wqu_sb[:, r, ds(m * 128, 128)], qlg[:, r, :],
                        start=(r == 0), stop=(r == RQ - 1),
                    )
                nc.tensor.matmul(
                    ps1, wqu_sb[:, r, ds(NH * D_NOPE, 128)], qlg[:, r, :],
                    start=(r == 0), stop=(r == RQ - 1),
                )
                nc.tensor.matmul(
                    ps2, wqu_sb[:, r, ds(NH * D_NOPE + 128, 128)], qlg[:, r, :],
                    start=(r == 0), stop=(r == RQ - 1),
                )
            for m in range(NH):
                nc.vector.tensor_copy(qn_sb[:, m, csl], qn_ps[m])
            qa = wb.tile([128, T], f32, tag="qa", bufs=1)
            qb = wb.tile([128, T], f32, tag="qb", bufs=1)
            nc.vector.tensor_mul(qa, ps1, cos_sb[:, csl])
            nc.vector.tensor_mul(qb, ps2, sin_sb[:, csl])
            y1 = wb.tile([128, T], bf16, tag="y1", bufs=2)
            nc.vector.tensor_sub(y1, qa, qb)
            qa2 = wb.tile([128, T], f32, tag="qa", bufs=1)
            qb2 = wb.tile([128, T], f32, tag="qb", bufs=1)
            nc.vector.tensor_mul(qa2, ps2, cos_sb[:, csl])
            nc.vector.tensor_mul(qb2, ps1, sin_sb[:, csl])
            y2 = wb.tile([128, T], bf16, tag="y2", bufs=2)
            nc.vector.tensor_add(y2, qa2, qb2)
            # assemble per-head [x1(32); x2(32)] rope layout, duplicated at
            # 64-127 (sync queue, idle by now)
            for h in range(NH):
                nc.sync.dma_start(out=qr2_sb[0:32, h, csl], in_=y1[ds(32 * h, 32), :])
                nc.sync.dma_start(out=qr2_sb[32:64, h, csl], in_=y2[ds(32 * h, 32), :])
                nc.sync.dma_start(out=qr2_sb[64:96, h, csl], in_=y1[ds(32 * h, 32), :])
                nc.sync.dma_start(out=qr2_sb[96:128, h, csl], in_=y2[ds(32 * h, 32), :])

        # ================= phase B + C: attention with interleaved out-proj =====
        wb.release()
        wc = tc.alloc_tile_pool(name="wc", bufs=2)

        norm_pend = []

        def drain_norm(stn):
            h_, qsl_, pv_, den_ = stn
            rec = wc.tile([128, QC], f32, tag="rec", bufs=2)
            nc.vector.reciprocal_approx_fast(out=rec, in_=den_)
            nc.vector.tensor_mul(at_sb[:, h_, qsl_], pv_, rec)

        for qc in range(NQC):
            qsl = ds(qc * QC, QC)
            nkc = 4 * qc + 4
            npair = nkc // 2
            for h in range(NH):
                pv = pv_ps.tile([128, QC], f32, tag="pv")
                den_ps = aux_ps.tile([128, QC], f32, tag="aux", name="den_ps")
                pend = []
                for t in range(npair):
                    kcA, kcB = 2 * t, 2 * t + 1
                    dA, dB = kcA - 4 * qc, kcB - 4 * qc
                    sct = sc_ps.tile([128, 1024], f32, tag="sc", name="sct_b")
                    nc.tensor.matmul(
                        sct[:, 0:512], kn_sb[:, h, ds(kcA * 128, 128)], qn_sb[:, h, qsl],
                        start=True, stop=False,
                    )
                    nc.tensor.matmul(
                        sct[:, 512:1024], kn_sb[:, h, ds(kcB * 128, 128)], qn_sb[:, h, qsl],
                        start=True, stop=False,
                    )
                    # the two K=64 rope matmuls land in disjoint row-groups -> concurrent
                    nc.tensor.matmul(
                        sct[:, 0:512], krope2_sb[0:64, ds(kcA * 128, 128)],
                        qr2_sb[0:64, h, qsl], start=False, stop=(dA < 0),
                    )
                    nc.tensor.matmul(
                        sct[:, 512:1024], krope2_sb[64:128, ds(kcB * 128, 128)],
                        qr2_sb[64:128, h, qsl], start=False, stop=(dB < 0),
                    )
                    if dA >= 0:
                        # causal mask: add -3e4 on invalid entries via identity matmul
                        pi = dA // 2
                        nc.tensor.matmul(
                            sct[:, 0:512], ident_sb[:], mask_sb[:, pi, 0:512],
                            start=False, stop=True,
                        )
                        nc.tensor.matmul(
                            sct[:, 512:1024], ident_sb[:], mask_sb[:, pi, 512:1024],
                            start=False, stop=True,
                        )
                    E = wc.tile([128, 1024], bf16, tag="E", bufs=6)
                    nc.scalar.activation(E, sct, AF.Exp, scale=SCALE)
                    Eh = wc.tile([128, QC], bf16, tag="Eh", bufs=4)
                    nc.vector.tensor_add(Eh, E[:, 0:512], E[:, 512:1024])
                    pend.append((t, E, Eh))
                    if len(pend) > (2 if npair >= 3 else 1):
                        pt, pE, pEh = pend.pop(0)
                        nc.tensor.matmul(
                            pv, v_sb[:, 2 * pt, ds(h * V_DIM, V_DIM)], pE[:, 0:512],
                            start=(pt == 0), stop=False,
                        )
                        nc.tensor.matmul(
                            pv, v_sb[:, 2 * pt + 1, ds(h * V_DIM, V_DIM)], pE[:, 512:1024],
                            start=False, stop=False,
                        )
                        nc.tensor.matmul(
                            den_ps, ones_sb[:], pEh,
                            start=(pt == 0), stop=False,
                        )
                while pend:
                    pt, pE, pEh = pend.pop(0)
                    last = not pend
                    nc.tensor.matmul(
                        pv, v_sb[:, 2 * pt, ds(h * V_DIM, V_DIM)], pE[:, 0:512],
                        start=(pt == 0), stop=False,
                    )
                    nc.tensor.matmul(
                        pv, v_sb[:, 2 * pt + 1, ds(h * V_DIM, V_DIM)], pE[:, 512:1024],
                        start=False, stop=last,
                    )
                    nc.tensor.matmul(
                        den_ps, ones_sb[:], pEh,
                        start=(pt == 0), stop=last,
                    )
                norm_pend.append((h, qsl, pv, den_ps))
                if len(norm_pend) > 1:
                    drain_norm(norm_pend.pop(0))
            while norm_pend:
                drain_norm(norm_pend.pop(0))
            # ---- out-projection for this qc's 4 token blocks ----
            for t16 in range(qc * 4, qc * 4 + 4):
                o_row = wc.tile([128, HID], f16, tag="ot", bufs=2)
                for n in range(HID // 512):
                    # rotate across sc halves AND pv tiles for a 6-deep psum
                    # rotation (hides the cast WAR)
                    if n < 2:
                        ps = sc_half(st, n)
                    else:
                        ps = pv_ps.tile([128, 512], f32, tag="pv", name="cps")
                    for f in range(NH):
                        nc.tensor.matmul(
                            ps, at_sb[:, f, ds(t16 * 128, 128)], wout_sb[:, f, ds(n * 512, 512)],
                            start=(f == 0), stop=(f == NH - 1),
                        )
                    nc.vector.tensor_copy(o_row[:, ds(n * 512, 512)], ps)
                nc.sync.dma_start(out=out_ap[ds(t16 * 128, 128), :], in_=o_row)

        wc.release()
        kvsb.release()
        w2.release()

    nc.compile()
    return nc


def get_nc():
    if "nc" not in _CACHE:
        _CACHE["nc"] = build_nc()
    return _CACHE["nc"]


def host_inputs(x, w_q_down, w_q_up, w_kv_down, kv_norm_w, w_kv_up, w_out):
    """Build the 8 per-core input shards (host-side prep, numpy only)."""
    bf = ml_dtypes.bfloat16
    x = np.asarray(x, np.float32)
    inv = 1.0 / THETA ** (np.arange(0, D_ROPE, 2, dtype=np.float64) / D_ROPE)
    ang = np.arange(S, dtype=np.float64)[:, None] * inv[None, :]      # (S, 32)
    cosq = np.ascontiguousarray(np.tile(np.cos(ang).T, (4, 1))).astype(bf)  # (128, S)
    sinq = np.ascontiguousarray(np.tile(np.sin(ang).T, (4, 1))).astype(bf)
    # additive causal masks for the diagonal-band chunks, paired (d, d+1)
    maskp = np.zeros((128, 2, 1024), np.float32)
    r = np.arange(128)[:, None]
    j = np.arange(512)[None, :]
    for d in range(4):
        maskp[:, d // 2, 512 * (d % 2) : 512 * (d % 2) + 512] = np.where(
            j >= 128 * d + r, 0.0, -30000.0
        )
    maskp = maskp.astype(bf)
    ones128 = np.ones((128, 128), bf)
    ident128 = np.eye(128, dtype=np.float32).astype(bf)
    wkv_eff = np.asarray(w_kv_up, np.float32) * np.asarray(kv_norm_w, np.float32)[:, None]

    def pmaj(w, *shape):
        # [K*128, N] -> partition-major [128, K, N] (-> optional extra reshape)
        kk = w.shape[0] // 128
        out = np.ascontiguousarray(w.reshape(kk, 128, w.shape[1]).transpose(1, 0, 2))
        return out.reshape(shape) if shape else out

    xT_bf = [np.ascontiguousarray(x[b].T).astype(bf) for b in range(B)]
    wqd_bf = np.asarray(w_q_down, np.float32).astype(bf)
    # wqd: [128, 3 rank-groups, 16 k-chunks, 512]
    wqd_pm = np.ascontiguousarray(
        wqd_bf.reshape(KH, 128, 3, 512).transpose(1, 2, 0, 3)
    )
    wkvd_pm = pmaj(np.asarray(w_kv_down, np.float32).astype(bf))
    wqu_f = np.asarray(w_q_up, np.float32)
    wout_f = np.asarray(w_out, np.float32)

    in_maps = []
    for ci in range(NCORES):
        b, hg = divmod(ci, 4)
        heads = list(range(NH * hg, NH * hg + NH))
        qu_cols = (
            [h * HEAD_DIM + j2 for h in heads for j2 in range(D_NOPE)]
            + [h * HEAD_DIM + D_NOPE + j2 for h in heads for j2 in range(32)]
            + [h * HEAD_DIM + D_NOPE + 32 + j2 for h in heads for j2 in range(32)]
        )
        kn_cols = [h * (D_NOPE + V_DIM) + j2 for h in heads for j2 in range(D_NOPE)]
        v_cols = [h * (D_NOPE + V_DIM) + D_NOPE + j2 for h in heads for j2 in range(V_DIM)]
        xq = np.ascontiguousarray(xT_bf[b][:, 512 * hg : 512 * (hg + 1)])
        in_maps.append(
            {
                "x": pmaj(xq),
                "cosl": np.ascontiguousarray(cosq[:, 512 * hg : 512 * (hg + 1)]),
                "sinl": np.ascontiguousarray(sinq[:, 512 * hg : 512 * (hg + 1)]),
                "wqd": wqd_pm,
                "wqu": pmaj(np.ascontiguousarray(wqu_f[:, qu_cols]).astype(bf)),
                "wkvd": wkvd_pm,
                "wkvuk": pmaj(np.ascontiguousarray(wkv_eff[:, kn_cols]).astype(bf)),
                "wkvuv": pmaj(np.ascontiguousarray(wkv_eff[:, v_cols]).astype(bf)),
                "wout": pmaj(
                    np.ascontiguousarray(
                        wout_f[NH * V_DIM * hg : NH * V_DIM * (hg + 1), :]
                    ).astype(bf)
                ),
                "cosq": cosq,
                "sinq": sinq,
                "maskp": maskp,
                "ones128": ones128,
                "ident128": ident128,
            }
        )
    return in_maps


def run(inputs, trace=False, trace_cores=None):
    from concourse.bass_utils import run_bass_kernel_spmd

    nc = get_nc()
    in_maps = host_inputs(**inputs)
    res = run_bass_kernel_spmd(
        nc,
        in_maps,
        core_ids=list(range(NCORES)),
        trace=trace,
        trace_cores=trace_cores,
    )
    out = np.zeros((B, S, HID), np.float32)
    for ci in range(NCORES):
        out[ci // 4] += res.results[ci]["out"].astype(np.float32)
    return out, res


def kernel(**inputs):
    out, _ = run(inputs, trace=False)
    return out
